# revision 52
# baseline (speedup 1.0000x reference)
"""Trainium2 Bass kernel for CareGptOssAttentionHF (MLA-style sliding-window
attention with sinks).

Sharding: sequence-parallel across 8 NeuronCores. Core c owns query rows
[c*256, (c+1)*256) and redundantly computes latent/K/V for its 768-row key
halo [c*256-512, c*256+256) — no collectives needed (window = 512).

v3 structure (v1 262us, v2 221us):
  * Every large input is host-pre-shuffled to a [128, N] layout whose
    per-partition bytes are contiguous in DRAM (8-16KB chunks), so DMAs
    run at line rate.
  * Phase order: latent -> k_nope -> V -> q-proj -> attention -> o-proj.
    kf/v live in fresh SBUF (pool opened first), so K/V assembly overlaps
    the Wq-gated q projection instead of serializing after it.
  * Packed 1536-col score layout per head; the two uniformly-dead
    (query-tile, key-block) pairs are never computed; causal/window edges
    are two static 128x128 triangles applied as 2 paired DVE multiplies.
  * Sequence-start (pad) keys: latent columns zeroed, and the PV "ones"
    column is the key-validity mask, so pad keys contribute exactly zero
    to numerator and denominator.
  * Softmax denominator reciprocal batched 8 heads per DVE op (the native
    reciprocal has ~1.4us fixed cost); numerators stashed in oat and
    normalized in place.
"""

import os
import sys

import numpy as np

if "/opt/trn_rl_repo" not in sys.path:
    sys.path.insert(0, "/opt/trn_rl_repo")

B, S, HID, H = 1, 2048, 2048, 16
NOPE = ROPE = 64
D = NOPE + ROPE  # 128
V = 64
R = 512
SW = 512
NCORES = 8
Q = S // NCORES  # 256 query rows per core
KH = Q + SW  # 768 halo key rows per core
NJB = KH // 128  # 6 key blocks
NIT = Q // 128  # 2 query tiles
SCALE = float(D) ** -0.5
PACK = 1408  # packed score cols, zero dead columns in the exp ranges:
# jb0@[0:128] (qtile0), jb1@[128:384], [384:512] pad (bank-0 fill),
# jb2@[512:768], jb3@[768:1024], jb4@[1024:1280], jb5@[1280:1408] (qtile1)
SLOT = {0: 0, 1: 128, 2: 512, 3: 768, 4: 1024, 5: 1280}
DEBUG = bool(int(os.environ.get("BASSDBG", "0")))

_CACHE = {}


def _build_program():
    import concourse.bass as bass
    import concourse.mybir as mybir
    from concourse import tile
    from contextlib import ExitStack

    f32 = mybir.dt.float32
    f16 = mybir.dt.float16
    AF = mybir.ActivationFunctionType

    nc = bass.Bass()

    # [128, N] host-pre-shuffled layouts: element [p, k*F + f] = orig[k*128+p, f]
    hsh_d = nc.dram_tensor("hshl", [128, 16 * SW], f16, kind="ExternalInput")
    hso_d = nc.dram_tensor("hsol", [128, 16 * Q], f16, kind="ExternalInput")
    wq_d = nc.dram_tensor("wql", [128, 16 * H * D], f16, kind="ExternalInput")
    wkva_d = nc.dram_tensor("wkval", [128, 16 * (R + ROPE)], f16, kind="ExternalInput")
    wkc_d = nc.dram_tensor("wkcl", [128, 4 * H * NOPE], f16, kind="ExternalInput")
    wvc_d = nc.dram_tensor("wvcl", [128, 4 * H * V], f16, kind="ExternalInput")
    wo_d = nc.dram_tensor("wol", [128, 8 * HID], f16, kind="ExternalInput")
    bq_d = nc.dram_tensor("bq", [128, 16], f32, kind="ExternalInput")
    bkva_d = nc.dram_tensor("bkva", [128, 5], f32, kind="ExternalInput")
    bo_d = nc.dram_tensor("bo", [128, HID], f16, kind="ExternalInput")
    tqc_d = nc.dram_tensor("trigq_cos", [128, Q], f16, kind="ExternalInput")
    tqs_d = nc.dram_tensor("trigq_sin", [128, Q], f16, kind="ExternalInput")
    tk_d = nc.dram_tensor("trigk", [128, 2, KH], f16, kind="ExternalInput")
    m1m2_d = nc.dram_tensor("m1m2", [128, 2, 128], f16, kind="ExternalInput")
    kmr_d = nc.dram_tensor("kmr", [128, SW], f16, kind="ExternalInput")
    kmk_d = nc.dram_tensor("kmk", [128, NJB], f16, kind="ExternalInput")
    esink_d = nc.dram_tensor("esink", [128, H], f32, kind="ExternalInput")
    out_d = nc.dram_tensor("out", [Q, HID], f32, kind="ExternalOutput")

    dbg = {}
    if DEBUG:
        dbg["lat"] = nc.dram_tensor("dbg_lat", [128, 4, KH], f16, kind="ExternalOutput")
        dbg["lat4"] = nc.dram_tensor("dbg_lat4", [64, KH], f16, kind="ExternalOutput")
        dbg["q"] = nc.dram_tensor("dbg_q", [128, H, Q], f16, kind="ExternalOutput")
        dbg["kf"] = nc.dram_tensor("dbg_kf", [128, H, KH], f16, kind="ExternalOutput")
        dbg["v"] = nc.dram_tensor("dbg_v", [128, NJB, H * 2 * V], f16, kind="ExternalOutput")
        dbg["pr"] = nc.dram_tensor("dbg_pr", [128, H, PACK], f16, kind="ExternalOutput")
        dbg["oat"] = nc.dram_tensor("dbg_oat", [128, 8, Q], f16, kind="ExternalOutput")

    with tile.TileContext(nc) as tc, ExitStack() as ctx:
        const = ctx.enter_context(tc.tile_pool(name="const", bufs=1))

        # ---- resident const tiles ----
        hso = const.tile([128, 16, Q], f16)
        wkc = const.tile([128, 4, H * NOPE], f16)
        wvc = const.tile([128, 4, H * V], f16)
        wo_sb = const.tile([128, 8, HID], f16)  # DMA deferred until after Wq
        bq_sb = const.tile([128, 16], f32)
        bkva_sb = const.tile([128, 5], f32)
        m1m2 = const.tile([128, 2, 128], f16)
        kmk = const.tile([128, NJB], f16)
        esink_sb = const.tile([128, H], f32)

        qT = const.tile([128, H, Q], f16)
        lat4b = const.tile([128, KH], f16)  # rows 64:128 = k_rope (RoPE'd)
        oat = const.tile([128, 8, Q], f16)
        # const (never-released) region: the output DMAs must carry only
        # their DVE-producer wait, so out_sb can't sit on reused space
        out_sb = const.tile([128, NIT, HID], f16)

        # kf/v/bo in fresh space, pool opened before the (short-lived) latent
        # pools so K/V assembly never waits on a region release
        kvp = ctx.enter_context(tc.tile_pool(name="kv", bufs=1))
        kf = kvp.tile([128, H, KH], f16)
        v_sb = kvp.tile([128, NJB, H * 2 * V], f16)
        bo_sb = kvp.tile([128, HID], f16)
        midp = ctx.enter_context(tc.tile_pool(name="mid", bufs=1))
        latbf = midp.tile([128, 4, KH], f16)
        # first quarter of Wq (k=0..3) in fresh space so its DMA starts
        # immediately; the rest reuses the latent-weights region
        wqp1 = ctx.enter_context(tc.tile_pool(name="wqp1", bufs=1))
        wq_g0 = wqp1.tile([128, 4, H * D], f16)
        tqc = wqp1.tile([128, Q], f16)
        tqs = wqp1.tile([128, Q], f16)

        def bc(ap, n):
            # broadcast a [P, F] AP to [P, n, F] via a step-0 middle dim
            return bass.AP(ap.tensor, ap.offset, [ap.ap[0], [0, n], ap.ap[1]])

        def bcf(col, n):
            # broadcast a [P, 1] column AP to [P, n] via a step-0 free dim
            return bass.AP(col.tensor, col.offset, [col.ap[0], [0, n]])

        def pair(t2d, col0, stride, width):
            # [P, 2, width] view of a [P, F] tile: cols {col0, col0+stride}
            s = t2d[:, col0 : col0 + width]
            return bass.AP(s.tensor, s.offset, [s.ap[0], [stride, 2], s.ap[1]])

        # ---- phase L: latent projection ----
        with tc.tile_pool(name="wkvap", bufs=1) as wkvap:
            wkva = wkvap.tile([128, 16, R + ROPE], f16)
            hsh = wkvap.tile([128, 16, SW], f16)
            tk = wkvap.tile([128, 2, KH], f16)  # rows 64:128 = key cos/sin
            kmr = wkvap.tile([128, SW], f16)
            rotk = wkvap.tile([128, KH], f16)  # rows 64:128 scratch
            for g in range(2):
                nc.sync.dma_start(
                    wkva[:, g * 8 : (g + 1) * 8, :],
                    wkva_d[:, g * 8 * 576 : (g + 1) * 8 * 576].rearrange(
                        "p (k f) -> p k f", f=576
                    ),
                )
                nc.sync.dma_start(
                    hsh[:, g * 8 : (g + 1) * 8, :],
                    hsh_d[:, g * 8 * SW : (g + 1) * 8 * SW].rearrange(
                        "p (k f) -> p k f", f=SW
                    ),
                )
            nc.sync.dma_start(hso[:], hso_d[:].rearrange("p (k f) -> p k f", f=Q))
            nc.sync.dma_start(bkva_sb[:], bkva_d[:])
            nc.sync.dma_start(tk[:], tk_d[:])
            nc.sync.dma_start(kmr[:], kmr_d[:])
            nc.sync.dma_start(kmk[:], kmk_d[:])
            nc.sync.dma_start(bq_sb[:], bq_d[:])
            nc.sync.dma_start(m1m2[:], m1m2_d[:])
            nc.sync.dma_start(esink_sb[:], esink_d[:])
            nc.sync.dma_start(tqc[:], tqc_d[:])
            nc.sync.dma_start(tqs[:], tqs_d[:])
            # Wq first quarter streams in right behind the latent inputs
            nc.sync.dma_start(wq_g0[:], wq_d[:, 0 : 4 * H * D])
            nc.sync.dma_start(wkc[:], wkc_d[:].rearrange("p (k f) -> p k f", f=H * NOPE))
            nc.sync.dma_start(wvc[:], wvc_d[:].rearrange("p (k f) -> p k f", f=H * V))

            with tc.tile_pool(name="pslat", bufs=1, space="PSUM") as pslatp:
                pslat = [
                    pslatp.tile([128, KH], f32, tag=f"pslat{m}", name=f"pslat{m}")
                    for m in range(4)
                ]
                for k in range(16):
                    for m in range(4):
                        nc.tensor.matmul(
                            pslat[m][:, 0:SW],
                            lhsT=wkva[:, k, m * 128 : (m + 1) * 128],
                            rhs=hsh[:, k, :],
                            start=(k == 0),
                            stop=(k == 15),
                        )
                        nc.tensor.matmul(
                            pslat[m][:, SW:KH],
                            lhsT=wkva[:, k, m * 128 : (m + 1) * 128],
                            rhs=hso[:, k, :],
                            start=(k == 0),
                            stop=(k == 15),
                        )
                ps4 = pslatp.tile([64, KH], f32, tag="pslat0")
                for k in range(16):
                    nc.tensor.matmul(
                        ps4[:, 0:SW],
                        lhsT=wkva[:, k, 512:576],
                        rhs=hsh[:, k, :],
                        start=(k == 0),
                        stop=(k == 15),
                    )
                    nc.tensor.matmul(
                        ps4[:, SW:KH],
                        lhsT=wkva[:, k, 512:576],
                        rhs=hso[:, k, :],
                        start=(k == 0),
                        stop=(k == 15),
                    )
                for m in range(4):
                    nc.vector.tensor_add(
                        latbf[:, m, :], pslat[m][:], bcf(bkva_sb[:, m : m + 1], KH)
                    )
                    # zero pad-key columns (sequence start): kills k_nope & v
                    nc.vector.tensor_mul(
                        latbf[:, m, 0:SW], latbf[:, m, 0:SW], kmr[:]
                    )
                # k_rope into partitions 64:128 (partition-shifted DVE add)
                # rope features 512:576 sit in rows 0:64 of bkva column 4
                nc.vector.tensor_add(
                    lat4b[64:128, :], ps4[:], bcf(bkva_sb[0:64, 4:5], KH)
                )
                nc.vector.tensor_mul(
                    lat4b[64:128, 0:SW], lat4b[64:128, 0:SW], kmr[64:128, :]
                )

            # ---- RoPE on k_rope (rows 64:128 of lat4b; tk rows 64:128) ----
            nc.vector.tensor_copy(rotk[64:96, :], lat4b[96:128, :])
            nc.vector.tensor_copy(rotk[96:128, :], lat4b[64:96, :])
            nc.vector.tensor_mul(lat4b[64:128, :], lat4b[64:128, :], tk[64:128, 0, :])
            nc.vector.tensor_mul(rotk[64:96, :], rotk[64:96, :], tk[64:96, 1, :])
            nc.vector.tensor_sub(lat4b[64:96, :], lat4b[64:96, :], rotk[64:96, :])
            nc.vector.tensor_mul(rotk[96:128, :], rotk[96:128, :], tk[96:128, 1, :])
            nc.vector.tensor_add(lat4b[96:128, :], lat4b[96:128, :], rotk[96:128, :])

        # ---- phase Q: q projection (before K/V assembly, so the rope-q and
        # kf/v copies on ACT/DVE hide under the KN/V matmul stream) ----
        # All 16 feature-major [128, 256] accumulators live as 8 [128, 512]
        # bank tiles (two halves each). has_written is cleared once per bank
        # (k==0, even m); the odd-m k==0 matmul then overwrites its untouched
        # half and every k>0 matmul accumulates.
        with tc.tile_pool(name="wqp2", bufs=1) as wqp2, tc.tile_pool(
            name="psq", bufs=1, space="PSUM"
        ) as psqp:
            wq_hi = [
                wqp2.tile([128, 4, H * D], f16, name=f"wq{1 + g}") for g in range(3)
            ]
            rotq = wqp2.tile([128, 8, Q], f16)
            for g in range(3):
                nc.sync.dma_start(
                    wq_hi[g][:],
                    wq_d[:, (1 + g) * 4 * H * D : (2 + g) * 4 * H * D],
                )
            # Wo + bo: queued behind Wq on the sync DGE
            for g in range(2):
                nc.sync.dma_start(
                    wo_sb[:, g * 4 : (g + 1) * 4, :],
                    wo_d[:, g * 4 * HID : (g + 1) * 4 * HID],
                )
            nc.sync.dma_start(bo_sb[:], bo_d[:])

            psq = [
                psqp.tile([128, 512], f32, tag=f"psq{i}", name=f"psq{i}")
                for i in range(8)
            ]
            mm_k0 = {}
            for k in range(16):
                wq_t = wq_g0 if k < 4 else wq_hi[k // 4 - 1]
                for m in range(16):
                    mm = nc.tensor.matmul(
                        psq[m // 2][:, (m % 2) * 256 : (m % 2) * 256 + 256],
                        lhsT=wq_t[:, k % 4, m * 128 : (m + 1) * 128],
                        rhs=hso[:, k, :],
                        start=(k == 0 and m % 2 == 0),
                        stop=(k == 15),
                        skip_group_check=True,
                    )
                    if k == 0:
                        mm_k0[m] = mm
                        if m % 2 == 1:
                            # the even-m k==0 matmul's start=True clears the
                            # whole bank's has_written bits; the odd-m k==0
                            # matmul must run after it (order-only dep)
                            tile.add_dep_helper(
                                mm.ins,
                                mm_k0[m - 1].ins,
                                sync=False,
                                reason="psq half-bank: odd k0 after even k0",
                            )
            for m in range(16):
                nc.vector.tensor_add(
                    qT[:, m, :],
                    psq[m // 2][:, (m % 2) * 256 : (m % 2) * 256 + 256],
                    bcf(bq_sb[:, m : m + 1], Q),
                )

            # ---- RoPE on q (rows 64:128), 4-head chunks; rotate-copies on
            # ACT so early heads unblock attention quickly ----
            for hb in range(4):
                hs_ = slice(hb * 4, (hb + 1) * 4)
                rq = rotq[:, (hb % 2) * 4 : (hb % 2) * 4 + 4, :]
                nc.scalar.copy(rq[96:128, :, :], qT[64:96, hs_, :])
                nc.scalar.copy(rq[64:96, :, :], qT[96:128, hs_, :])
                nc.vector.tensor_mul(
                    qT[64:128, hs_, :], qT[64:128, hs_, :], bc(tqc[64:128, :], 4)
                )
                nc.vector.tensor_mul(
                    rq[64:128, :, :], rq[64:128, :, :], bc(tqs[64:128, :], 4)
                )
                nc.vector.tensor_sub(
                    qT[64:96, hs_, :], qT[64:96, hs_, :], rq[64:96, :, :]
                )
                nc.vector.tensor_add(
                    qT[96:128, hs_, :], qT[96:128, hs_, :], rq[96:128, :, :]
                )

        # "ones" columns of v = key-validity mask, one 4D broadcast copy
        vones_view = v_sb[:].rearrange("p j (h d) -> p j h d", d=2 * V)[
            :, :, :, V : 2 * V
        ]
        kap = kmk[:]
        kmk_bcast = bass.AP(
            kap.tensor, kap.offset, [kap.ap[0], [1, NJB], [0, H], [0, V]]
        )
        nc.gpsimd.tensor_copy(vones_view, kmk_bcast)

        # ---- phase KN: k_nope into kf rows 0:64 (ACT), rope rows broadcast
        # into rows 64:128 (DVE; a K-split pair of row-group matmuls
        # accumulating into one PSUM region hard-crashes the device, so the
        # shared rope rows must be materialized per head) ----
        with tc.tile_pool(name="pskn", bufs=4, space="PSUM") as psknp:
            for m in range(8):
                ps = psknp.tile([128, KH], f32, tag="pskn")
                for k in range(4):
                    for n0, n1 in ((0, 512), (512, KH)):
                        nc.tensor.matmul(
                            ps[:, n0:n1],
                            lhsT=wkc[:, k, m * 128 : (m + 1) * 128],
                            rhs=latbf[:, k, n0:n1],
                            start=(k == 0),
                            stop=(k == 3),
                        )
                # PSUM->SBUF halves split across ACT and DVE; the shared rope
                # rows ride the (otherwise idle) DMA engines, SBUF->SBUF
                nc.scalar.copy(kf[0:64, 2 * m, :], ps[0:64, :])
                nc.vector.tensor_copy(kf[0:64, 2 * m + 1, :], ps[64:128, :])
                nc.sync.dma_start(kf[64:128, 2 * m, :], lat4b[64:128, :])
                nc.sync.dma_start(kf[64:128, 2 * m + 1, :], lat4b[64:128, :])

        # ---- phase V: V (key-major) ----
        with tc.tile_pool(name="psv", bufs=2, space="PSUM") as psvp:
            for jb in range(NJB):
                ps = psvp.tile([128, H * V], f32, tag="psv")
                for k in range(4):
                    for n0, n1 in ((0, 512), (512, 1024)):
                        nc.tensor.matmul(
                            ps[:, n0:n1],
                            lhsT=latbf[:, k, jb * 128 : (jb + 1) * 128],
                            rhs=wvc[:, k, n0:n1],
                            start=(k == 0),
                            stop=(k == 3),
                        )
                vview = v_sb[:, jb, :].rearrange("p (h d) -> p h d", d=2 * V)
                ps_view = ps[:].rearrange("p (h d) -> p h d", d=V)
                if jb % 2 == 0:
                    nc.scalar.copy(vview[:, :, 0:V], ps_view)
                else:
                    nc.vector.tensor_copy(vview[:, :, 0:V], ps_view)

        # ---- phase A: attention, packed 1536-col score layout ----
        # Score slots (cols): jb0 -> [0:128] (query tile 0 only; the it1 half
        # is uniformly outside the window), jb1..4 -> [jb*256 : jb*256+256]
        # (both query tiles), jb5 -> [1408:1536] (query tile 1 only).
        # Each block's score = two row-group-concurrent K=64 matmuls:
        # nope (kfn, rows 0:64) + shared rope (lat4b, rows 64:128).
        # Static masks: M2 (p>c, window edge) on cols {0,384}; M1 (p<=c,
        # causal edge) on cols {1024,1408} — identical for every core/head.
        probs_tiles = {}
        with tc.tile_pool(name="att_sbuf", bufs=2) as attp, tc.tile_pool(
            name="att_psum", bufs=2, space="PSUM"
        ) as attps, tc.tile_pool(name="stat", bufs=2) as statp:

            def sc_block(ps_s, h, jb, c0, q0, qn):
                return nc.tensor.matmul(
                    ps_s[:, c0 : c0 + qn],
                    lhsT=kf[:, h, jb * 128 : (jb + 1) * 128],
                    rhs=qT[:, h, q0 : q0 + qn],
                    start=True,
                    stop=True,
                )

            def emit_scores(h):
                ps_s = attps.tile([128, PACK], f32, tag="ps_s")
                sc_block(ps_s, h, 0, SLOT[0], 0, 128)
                for jb in range(1, 5):
                    sc_block(ps_s, h, jb, SLOT[jb], 0, 256)
                sc_block(ps_s, h, 5, SLOT[5], 128, 128)
                pr = attp.tile([128, PACK], f16, tag="pr", bufs=3)
                # exp over exactly the live columns, split at the pad gap so
                # the first op depends only on the first two score matmuls
                nc.scalar.activation(
                    pr[:, 0:384], ps_s[:, 0:384], AF.Exp, bias=0.0, scale=SCALE
                )
                nc.scalar.activation(
                    pr[:, 512:PACK], ps_s[:, 512:PACK], AF.Exp, bias=0.0, scale=SCALE
                )
                # window edges (M2): jb0 qtile0 @0, jb1 qtile1 @256;
                # causal edges (M1): jb4 qtile0 @1024, jb5 qtile1 @1280
                nc.vector.tensor_mul(
                    pair(pr, 0, 256, 128), pair(pr, 0, 256, 128), bc(m1m2[:, 1, :], 2)
                )
                nc.vector.tensor_mul(
                    pair(pr, 1024, 256, 128),
                    pair(pr, 1024, 256, 128),
                    bc(m1m2[:, 0, :], 2),
                )
                probs_tiles[h] = pr

            def emit_pv(h):
                pr = probs_tiles.pop(h)
                ps_o = attps.tile([128, Q], f32, tag="ps_o")
                nc.tensor.matmul(
                    ps_o[:, 0:128],
                    lhsT=v_sb[:, 0, h * 2 * V : (h + 1) * 2 * V],
                    rhs=pr[:, 0:128],
                    start=True,
                    stop=False,
                    skip_group_check=True,
                )
                for jb in range(1, 5):
                    nc.tensor.matmul(
                        ps_o[:],
                        lhsT=v_sb[:, jb, h * 2 * V : (h + 1) * 2 * V],
                        rhs=pr[:, SLOT[jb] : SLOT[jb] + 256],
                        start=False,
                        stop=False,
                        skip_group_check=True,
                    )
                nc.tensor.matmul(
                    ps_o[:, 128:256],
                    lhsT=v_sb[:, 5, h * 2 * V : (h + 1) * 2 * V],
                    rhs=pr[:, SLOT[5] : SLOT[5] + 128],
                    start=False,
                    stop=True,
                    skip_group_check=True,
                )
                # denominator for this head into the pair tile (even head in
                # partitions 0:64, odd in 64:128, mirroring oat's layout)
                base = (h % 2) * 64
                if h % 2 == 0:
                    pair_state["ds"] = statp.tile(
                        [128, Q], f32, tag="dsum", name=f"dsp{h}"
                    )
                dspair = pair_state["ds"]
                nc.vector.tensor_add(
                    dspair[base : base + 64, :],
                    ps_o[64:128, :],
                    bcf(esink_sb[base : base + 64, h : h + 1], Q),
                )
                ps_pair[h % 2] = ps_o
                if h % 2 == 1:
                    # pairwise normalize: rcp = exp(-ln(d)) on ACT (the DVE
                    # reciprocal costs ~6.4ns/element; the two table lookups
                    # are ~4x cheaper and ACT has the headroom here)
                    lnd = statp.tile([128, Q], f32, tag="lnd")
                    nc.scalar.activation(lnd[:], dspair[:], AF.Ln)
                    rcp = statp.tile([128, Q], f32, tag="rcp")
                    nc.scalar.activation(rcp[:], lnd[:], AF.Exp, scale=-1.0)
                    nc.vector.tensor_mul(
                        oat[0:64, (h - 1) // 2, :],
                        ps_pair[0][0:64, :],
                        rcp[0:64, :],
                    )
                    nc.vector.tensor_mul(
                        oat[64:128, h // 2, :], ps_pair[1][0:64, :], rcp[64:128, :]
                    )
                if DEBUG:
                    nc.sync.dma_start(dbg["pr"][:, h, :], pr[:])

            ps_pair = {}
            pair_state = {}
            emit_scores(0)
            emit_scores(1)
            for h in range(2, H):
                emit_scores(h)
                emit_pv(h - 2)
            emit_pv(H - 2)
            emit_pv(H - 1)

        # ---- phase O: output projection (i-major) + bias + store; query
        # tile 0 finishes (and its output DMAs start) while tile 1's matmuls
        # are still streaming ----
        with tc.tile_pool(name="psf", bufs=1, space="PSUM") as psfp:
            psf = [
                psfp.tile([128, 512], f32, tag=f"psf{i}", name=f"psf{i}")
                for i in range(8)
            ]
            for it in range(NIT):
                for k in range(8):
                    for n in range(4):
                        nc.tensor.matmul(
                            psf[it * 4 + n][:],
                            lhsT=oat[:, k, it * 128 : (it + 1) * 128],
                            rhs=wo_sb[:, k, n * 512 : (n + 1) * 512],
                            start=(k == 0),
                            stop=(k == 7),
                        )
                for n in range(4):
                    nc.vector.tensor_add(
                        out_sb[:, it, n * 512 : (n + 1) * 512],
                        psf[it * 4 + n][:],
                        bo_sb[:, n * 512 : (n + 1) * 512],
                    )
                    # SWDGE (casts f16 -> f32 inline): first (and only) DMA on
                    # each SW queue, so the ring entry carries one wait.
                    nc.gpsimd.dma_start(
                        out_d[it * 128 : (it + 1) * 128, n * 512 : (n + 1) * 512],
                        out_sb[:, it, n * 512 : (n + 1) * 512],
                    )

        if DEBUG:
            nc.sync.dma_start(dbg["lat"][:], latbf[:])
            nc.sync.dma_start(dbg["lat4"][:], lat4b[64:128, :])
            nc.sync.dma_start(dbg["q"][:], qT[:])
            nc.sync.dma_start(dbg["kf"][:], kf[:])
            nc.sync.dma_start(dbg["v"][:], v_sb[:])
            nc.sync.dma_start(dbg["oat"][:], oat[:])

    if not bool(int(os.environ.get("BASSNOSPLIT", "0"))):
        _split_multi_waits(nc, mybir)
    nc.finalize()
    return nc


def _split_multi_waits(nc, mybir):
    """The TPB ISA has a single embedded wait slot per instruction and this
    toolchain's walrus pass list has no wait-splitting pass ("Too many sync
    wait commands"). Hoist all-but-one wait of every multi-wait compute
    instruction into standalone same-engine EventSemaphore instructions
    placed immediately before it. HWDGE (SP/ACT-issued) DMAs are fair game
    too: their waits execute on the issuing sequencer before descriptor
    generation, so a preceding same-engine EventSemaphore is semantically
    identical. SWDGE (Pool) ring entries can't be split — assert those."""
    seq_ok = (mybir.InstEventSemaphore,)
    hwdge = (mybir.EngineType.SP, mybir.EngineType.Activation)
    n = 0
    for fn in nc.m.functions:
        for blk in fn.blocks:
            out = []
            for inst in blk.instructions:
                si = inst.sync_info
                if si is not None and len(si.on_wait) > 1 and not isinstance(inst, seq_ok):
                    if isinstance(inst, mybir.InstDMACopy) and inst.engine not in hwdge:
                        raise AssertionError(
                            f"DMA {inst.name} on {inst.engine} has "
                            f"{len(si.on_wait)} waits; SWDGE DMAs must carry "
                            "at most one"
                        )
                    for w in si.on_wait[:-1]:
                        n += 1
                        out.append(
                            mybir.InstEventSemaphore(
                                name=f"I-wsplit-{n}",
                                engine=inst.engine,
                                ins=[],
                                outs=[],
                                sync_info=mybir.SyncInfo(on_wait=[w], on_update=[]),
                            )
                        )
                    inst.sync_info = mybir.SyncInfo(
                        on_wait=[si.on_wait[-1]], on_update=si.on_update
                    )
                out.append(inst)
            blk.instructions = out
    return n


def _shuffle128(a):
    """[K*128, F] -> [128, K*F] with [p, k*F+f] = a[k*128+p, f]."""
    k = a.shape[0] // 128
    return np.ascontiguousarray(
        a.reshape(k, 128, a.shape[1]).transpose(1, 0, 2).reshape(128, -1)
    )


def prep_inputs(
    hidden_states, cos, sin, Wq, bq, Wo, bo, Wkva, bkva, w_kc, w_vc, sinks
):
    """Build the 8 per-core input dicts (numpy, fp16/fp32)."""
    f16 = np.float16
    hs = np.asarray(hidden_states, np.float32)[0]  # [S, HID]
    cos = np.asarray(cos, np.float32)[0]  # [S, ROPE]
    sin = np.asarray(sin, np.float32)[0]

    wqT = np.asarray(Wq, np.float32).T.astype(f16)
    wkvaT = np.asarray(Wkva, np.float32).T.astype(f16)
    wkc_p = np.asarray(w_kc, np.float32).transpose(2, 0, 1).reshape(R, H * NOPE).astype(f16)
    wvc_p = np.asarray(w_vc, np.float32).transpose(1, 0, 2).reshape(R, H * V).astype(f16)
    woT = np.asarray(Wo, np.float32).T.astype(f16)

    bq_t = np.ascontiguousarray(np.asarray(bq, np.float32).reshape(16, 128).T)
    bkva_pad = np.zeros(640, np.float32)
    bkva_pad[: R + ROPE] = np.asarray(bkva, np.float32)
    bkva_t = np.ascontiguousarray(bkva_pad.reshape(5, 128).T)
    bo_b = np.ascontiguousarray(
        np.broadcast_to(np.asarray(bo, np.float32), (128, HID))
    ).astype(f16)
    esink_b = np.ascontiguousarray(
        np.broadcast_to(np.exp(np.asarray(sinks, np.float32))[None, :], (128, H))
    )

    # static triangular edge masks: M1 = p<=c (causal), M2 = p>c (window)
    pp = np.arange(128)[:, None]
    cc = np.arange(128)[None, :]
    m1m2 = np.zeros((128, 2, 128), np.float32)
    m1m2[:, 0, :] = (pp <= cc).astype(np.float32)
    m1m2[:, 1, :] = (pp > cc).astype(np.float32)
    m1m2 = m1m2.astype(f16)

    hs_pad = np.zeros((SW + S, HID), np.float32)
    hs_pad[SW:] = hs

    shared = dict(
        wql=_shuffle128(wqT),
        wkval=_shuffle128(wkvaT),
        wkcl=_shuffle128(wkc_p),
        wvcl=_shuffle128(wvc_p),
        wol=_shuffle128(woT),
        bq=bq_t, bkva=bkva_t, bo=bo_b, esink=esink_b, m1m2=m1m2,
    )

    in_maps = []
    for c in range(NCORES):
        g0 = c * Q
        hsTh_c = np.ascontiguousarray(hs_pad[g0 : g0 + SW].T).astype(f16)
        hsTo_c = np.ascontiguousarray(hs_pad[g0 + SW : g0 + KH].T).astype(f16)

        cq = cos[g0 : g0 + Q]  # [Q, 64]
        sq = sin[g0 : g0 + Q]
        tqc = np.zeros((128, Q), np.float32)
        tqs = np.zeros((128, Q), np.float32)
        tqc[64:96] = cq[:, 0:32].T
        tqc[96:128] = cq[:, 32:64].T
        tqs[64:96] = sq[:, 0:32].T
        tqs[96:128] = sq[:, 32:64].T

        kpos = np.clip(np.arange(g0 - SW, g0 + Q), 0, None)
        ck = cos[kpos]  # [KH, 64]
        sk = sin[kpos]
        tkk = np.zeros((128, 2, KH), np.float32)
        tkk[64:96, 0] = ck[:, 0:32].T
        tkk[96:128, 0] = ck[:, 32:64].T
        tkk[64:96, 1] = sk[:, 0:32].T
        tkk[96:128, 1] = sk[:, 32:64].T

        # key validity (sequence start padding)
        jg = (g0 - SW) + np.arange(KH)
        kmr_c = np.broadcast_to((jg[0:SW] >= 0).astype(np.float32), (128, SW))
        kmk_c = np.zeros((128, NJB), np.float32)
        for jb in range(NJB):
            kmk_c[:, jb] = (jg[jb * 128 : (jb + 1) * 128] >= 0).astype(np.float32)

        in_maps.append(
            dict(
                shared,
                hshl=_shuffle128(hsTh_c),
                hsol=_shuffle128(hsTo_c),
                trigq_cos=tqc.astype(f16),
                trigq_sin=tqs.astype(f16),
                trigk=tkk.astype(f16),
                kmr=np.ascontiguousarray(kmr_c).astype(f16),
                kmk=kmk_c.astype(f16),
            )
        )
    return in_maps


def get_program():
    if "nc" not in _CACHE:
        _CACHE["nc"] = _build_program()
    return _CACHE["nc"]


def run(in_maps, **kw):
    from concourse.bass_utils import run_bass_kernel_spmd

    nc = get_program()
    return run_bass_kernel_spmd(nc, in_maps, list(range(NCORES)), **kw)


def kernel(**inputs):
    in_maps = prep_inputs(**inputs)
    res = run(in_maps)
    out = np.concatenate([res.results[c]["out"] for c in range(NCORES)], axis=0)
    return out.reshape(B, S, HID).astype(np.float32)


# revision 63
# speedup vs baseline: 1.0796x; 1.0796x over previous
"""Trainium2 Bass kernel for CareGptOssAttentionHF (MLA-style sliding-window
attention with sinks).

Sharding: sequence-parallel across 8 NeuronCores. Core c owns query rows
[c*256, (c+1)*256) and redundantly computes latent/K/V for its 768-row key
halo [c*256-512, c*256+256) — no collectives needed (window = 512).

v3 structure (v1 262us, v2 221us):
  * Every large input is host-pre-shuffled to a [128, N] layout whose
    per-partition bytes are contiguous in DRAM (8-16KB chunks), so DMAs
    run at line rate.
  * Phase order: latent -> k_nope -> V -> q-proj -> attention -> o-proj.
    kf/v live in fresh SBUF (pool opened first), so K/V assembly overlaps
    the Wq-gated q projection instead of serializing after it.
  * Packed 1536-col score layout per head; the two uniformly-dead
    (query-tile, key-block) pairs are never computed; causal/window edges
    are two static 128x128 triangles applied as 2 paired DVE multiplies.
  * Sequence-start (pad) keys: latent columns zeroed, and the PV "ones"
    column is the key-validity mask, so pad keys contribute exactly zero
    to numerator and denominator.
  * Softmax denominator reciprocal batched 8 heads per DVE op (the native
    reciprocal has ~1.4us fixed cost); numerators stashed in oat and
    normalized in place.
"""

import os
import sys

import numpy as np

if "/opt/trn_rl_repo" not in sys.path:
    sys.path.insert(0, "/opt/trn_rl_repo")

B, S, HID, H = 1, 2048, 2048, 16
NOPE = ROPE = 64
D = NOPE + ROPE  # 128
V = 64
R = 512
SW = 512
NCORES = 8
Q = S // NCORES  # 256 query rows per core
KH = Q + SW  # 768 halo key rows per core
NJB = KH // 128  # 6 key blocks
NIT = Q // 128  # 2 query tiles
SCALE = float(D) ** -0.5
PACK = 1536  # packed score cols: 6 blocks x 256-col slots
DEBUG = bool(int(os.environ.get("BASSDBG", "0")))

_CACHE = {}


def _build_program():
    import concourse.bass as bass
    import concourse.mybir as mybir
    from concourse import tile
    from contextlib import ExitStack

    f32 = mybir.dt.float32
    f16 = mybir.dt.float16
    AF = mybir.ActivationFunctionType

    nc = bass.Bass()

    # [128, N] host-pre-shuffled layouts: element [p, k*F + f] = orig[k*128+p, f]
    hsh_d = nc.dram_tensor("hshl", [128, 16 * SW], f16, kind="ExternalInput")
    hso_d = nc.dram_tensor("hsol", [128, 16 * Q], f16, kind="ExternalInput")
    wq_d = nc.dram_tensor("wql", [128, 16 * H * D], f16, kind="ExternalInput")
    wkva_d = nc.dram_tensor("wkval", [128, 16 * (R + ROPE)], f16, kind="ExternalInput")
    wkc_d = nc.dram_tensor("wkcl", [128, 4 * H * NOPE], f16, kind="ExternalInput")
    wvc_d = nc.dram_tensor("wvcl", [128, 4 * H * V], f16, kind="ExternalInput")
    wo_d = nc.dram_tensor("wol", [128, 8 * HID], f16, kind="ExternalInput")
    bq_d = nc.dram_tensor("bq", [128, 16], f32, kind="ExternalInput")
    bkva_d = nc.dram_tensor("bkva", [128, 5], f32, kind="ExternalInput")
    bo_d = nc.dram_tensor("bo", [128, HID], f16, kind="ExternalInput")
    tqc_d = nc.dram_tensor("trigq_cos", [128, Q], f16, kind="ExternalInput")
    tqs_d = nc.dram_tensor("trigq_sin", [128, Q], f16, kind="ExternalInput")
    tk_d = nc.dram_tensor("trigk", [128, 2, KH], f16, kind="ExternalInput")
    m1m2_d = nc.dram_tensor("m1m2", [128, 2, 128], f16, kind="ExternalInput")
    kmr_d = nc.dram_tensor("kmr", [128, SW], f16, kind="ExternalInput")
    kmk_d = nc.dram_tensor("kmk", [128, NJB], f16, kind="ExternalInput")
    esink_d = nc.dram_tensor("esink", [128, H], f32, kind="ExternalInput")
    out_d = nc.dram_tensor("out", [Q, HID], f32, kind="ExternalOutput")

    dbg = {}
    if DEBUG:
        dbg["lat"] = nc.dram_tensor("dbg_lat", [128, 4, KH], f16, kind="ExternalOutput")
        dbg["lat4"] = nc.dram_tensor("dbg_lat4", [64, KH], f16, kind="ExternalOutput")
        dbg["q"] = nc.dram_tensor("dbg_q", [128, H, Q], f16, kind="ExternalOutput")
        dbg["kf"] = nc.dram_tensor("dbg_kf", [128, H, KH], f16, kind="ExternalOutput")
        dbg["v"] = nc.dram_tensor("dbg_v", [128, NJB, H * 2 * V], f16, kind="ExternalOutput")
        dbg["pr"] = nc.dram_tensor("dbg_pr", [128, H, PACK], f16, kind="ExternalOutput")
        dbg["oat"] = nc.dram_tensor("dbg_oat", [128, 8, Q], f16, kind="ExternalOutput")

    with tile.TileContext(nc) as tc, ExitStack() as ctx:
        const = ctx.enter_context(tc.tile_pool(name="const", bufs=1))

        # ---- resident const tiles ----
        hso = const.tile([128, 16, Q], f16)
        wkc = const.tile([128, 4, H * NOPE], f16)
        wvc = const.tile([128, 4, H * V], f16)
        wo_sb = const.tile([128, 8, HID], f16)  # DMA deferred until after Wq
        bq_sb = const.tile([128, 16], f32)
        bkva_sb = const.tile([128, 5], f32)
        m1m2 = const.tile([128, 2, 128], f16)
        kmk = const.tile([128, NJB], f16)
        esink_sb = const.tile([128, H], f32)

        qT = const.tile([128, H, Q], f16)
        lat4b = const.tile([128, KH], f16)  # rows 64:128 = k_rope (RoPE'd)
        oat = const.tile([128, 8, Q], f16)
        # const (never-released) region: the output DMAs must carry only
        # their DVE-producer wait, so out_sb can't sit on reused space
        out_sb = const.tile([128, NIT, HID], f16)

        # kf/v/bo in fresh space, pool opened before the (short-lived) latent
        # pools so K/V assembly never waits on a region release
        kvp = ctx.enter_context(tc.tile_pool(name="kv", bufs=1))
        kf = kvp.tile([128, H, KH], f16)
        v_sb = kvp.tile([128, NJB, H * 2 * V], f16)
        bo_sb = kvp.tile([128, HID], f16)
        midp = ctx.enter_context(tc.tile_pool(name="mid", bufs=1))
        latbf = midp.tile([128, 4, KH], f16)
        # first quarter of Wq (k=0..3) in fresh space so its DMA starts
        # immediately; the rest reuses the latent-weights region
        wqp1 = ctx.enter_context(tc.tile_pool(name="wqp1", bufs=1))
        wq_g0 = wqp1.tile([128, 4, H * D], f16)
        tqc = wqp1.tile([128, Q], f16)
        tqs = wqp1.tile([128, Q], f16)

        def bc(ap, n):
            # broadcast a [P, F] AP to [P, n, F] via a step-0 middle dim
            return bass.AP(ap.tensor, ap.offset, [ap.ap[0], [0, n], ap.ap[1]])

        def bcf(col, n):
            # broadcast a [P, 1] column AP to [P, n] via a step-0 free dim
            return bass.AP(col.tensor, col.offset, [col.ap[0], [0, n]])

        def pair(t2d, col0, stride, width):
            # [P, 2, width] view of a [P, F] tile: cols {col0, col0+stride}
            s = t2d[:, col0 : col0 + width]
            return bass.AP(s.tensor, s.offset, [s.ap[0], [stride, 2], s.ap[1]])

        # ---- phase L: latent projection ----
        with tc.tile_pool(name="wkvap", bufs=1) as wkvap:
            wkva = wkvap.tile([128, 16, R + ROPE], f16)
            hsh = wkvap.tile([128, 16, SW], f16)
            tk = wkvap.tile([128, 2, KH], f16)  # rows 64:128 = key cos/sin
            kmr = wkvap.tile([128, SW], f16)
            rotk = wkvap.tile([128, KH], f16)  # rows 64:128 scratch
            # interleave wkva/hsh/hso by 4-ktile groups: the first latent
            # matmuls need (wkva[k], hsh[k], hso[k]) together, so the PE
            # starts ~3us earlier than with whole-tensor-at-a-time loads
            for g in range(4):
                nc.sync.dma_start(
                    wkva[:, g * 4 : (g + 1) * 4, :],
                    wkva_d[:, g * 4 * 576 : (g + 1) * 4 * 576].rearrange(
                        "p (k f) -> p k f", f=576
                    ),
                )
                nc.sync.dma_start(
                    hsh[:, g * 4 : (g + 1) * 4, :],
                    hsh_d[:, g * 4 * SW : (g + 1) * 4 * SW].rearrange(
                        "p (k f) -> p k f", f=SW
                    ),
                )
                nc.sync.dma_start(
                    hso[:, g * 4 : (g + 1) * 4, :],
                    hso_d[:, g * 4 * Q : (g + 1) * 4 * Q].rearrange(
                        "p (k f) -> p k f", f=Q
                    ),
                )
            nc.sync.dma_start(bkva_sb[:], bkva_d[:])
            nc.sync.dma_start(tk[:], tk_d[:])
            nc.sync.dma_start(kmr[:], kmr_d[:])
            nc.sync.dma_start(kmk[:], kmk_d[:])
            nc.sync.dma_start(bq_sb[:], bq_d[:])
            nc.sync.dma_start(m1m2[:], m1m2_d[:])
            nc.sync.dma_start(esink_sb[:], esink_d[:])
            nc.sync.dma_start(tqc[:], tqc_d[:])
            nc.sync.dma_start(tqs[:], tqs_d[:])
            # Wq first quarter streams in right behind the latent inputs
            nc.sync.dma_start(wq_g0[:], wq_d[:, 0 : 4 * H * D])
            nc.sync.dma_start(wkc[:], wkc_d[:].rearrange("p (k f) -> p k f", f=H * NOPE))
            nc.sync.dma_start(wvc[:], wvc_d[:].rearrange("p (k f) -> p k f", f=H * V))

            with tc.tile_pool(name="pslat", bufs=1, space="PSUM") as pslatp:
                pslat = [
                    pslatp.tile([128, KH], f32, tag=f"pslat{m}", name=f"pslat{m}")
                    for m in range(4)
                ]
                for k in range(16):
                    for m in range(4):
                        nc.tensor.matmul(
                            pslat[m][:, 0:SW],
                            lhsT=wkva[:, k, m * 128 : (m + 1) * 128],
                            rhs=hsh[:, k, :],
                            start=(k == 0),
                            stop=(k == 15),
                        )
                        nc.tensor.matmul(
                            pslat[m][:, SW:KH],
                            lhsT=wkva[:, k, m * 128 : (m + 1) * 128],
                            rhs=hso[:, k, :],
                            start=(k == 0),
                            stop=(k == 15),
                        )
                ps4 = pslatp.tile([64, KH], f32, tag="pslat0")
                for k in range(16):
                    nc.tensor.matmul(
                        ps4[:, 0:SW],
                        lhsT=wkva[:, k, 512:576],
                        rhs=hsh[:, k, :],
                        start=(k == 0),
                        stop=(k == 15),
                    )
                    nc.tensor.matmul(
                        ps4[:, SW:KH],
                        lhsT=wkva[:, k, 512:576],
                        rhs=hso[:, k, :],
                        start=(k == 0),
                        stop=(k == 15),
                    )
                for m in range(4):
                    nc.vector.tensor_add(
                        latbf[:, m, :], pslat[m][:], bcf(bkva_sb[:, m : m + 1], KH)
                    )
                    # zero pad-key columns (sequence start): kills k_nope & v
                    nc.vector.tensor_mul(
                        latbf[:, m, 0:SW], latbf[:, m, 0:SW], kmr[:]
                    )
                # k_rope into partitions 64:128 (partition-shifted DVE add)
                # rope features 512:576 sit in rows 0:64 of bkva column 4
                nc.vector.tensor_add(
                    lat4b[64:128, :], ps4[:], bcf(bkva_sb[0:64, 4:5], KH)
                )
                nc.vector.tensor_mul(
                    lat4b[64:128, 0:SW], lat4b[64:128, 0:SW], kmr[64:128, :]
                )

            # ---- RoPE on k_rope (rows 64:128 of lat4b; tk rows 64:128) ----
            nc.vector.tensor_copy(rotk[64:96, :], lat4b[96:128, :])
            nc.vector.tensor_copy(rotk[96:128, :], lat4b[64:96, :])
            nc.vector.tensor_mul(lat4b[64:128, :], lat4b[64:128, :], tk[64:128, 0, :])
            nc.vector.tensor_mul(rotk[64:96, :], rotk[64:96, :], tk[64:96, 1, :])
            nc.vector.tensor_sub(lat4b[64:96, :], lat4b[64:96, :], rotk[64:96, :])
            nc.vector.tensor_mul(rotk[96:128, :], rotk[96:128, :], tk[96:128, 1, :])
            nc.vector.tensor_add(lat4b[96:128, :], lat4b[96:128, :], rotk[96:128, :])

        # ---- phase Q: q projection (before K/V assembly, so the rope-q and
        # kf/v copies on ACT/DVE hide under the KN/V matmul stream) ----
        # All 16 feature-major [128, 256] accumulators live as 8 [128, 512]
        # bank tiles (two halves each). has_written is cleared once per bank
        # (k==0, even m); the odd-m k==0 matmul then overwrites its untouched
        # half and every k>0 matmul accumulates.
        with tc.tile_pool(name="wqp2", bufs=1) as wqp2, tc.tile_pool(
            name="psq", bufs=1, space="PSUM"
        ) as psqp:
            wq_hi = [
                wqp2.tile([128, 4, H * D], f16, name=f"wq{1 + g}") for g in range(3)
            ]
            rotq = wqp2.tile([128, 8, Q], f16)
            for g in range(3):
                nc.sync.dma_start(
                    wq_hi[g][:],
                    wq_d[:, (1 + g) * 4 * H * D : (2 + g) * 4 * H * D],
                )
            # Wo + bo: queued behind Wq on the sync DGE
            for g in range(2):
                nc.sync.dma_start(
                    wo_sb[:, g * 4 : (g + 1) * 4, :],
                    wo_d[:, g * 4 * HID : (g + 1) * 4 * HID],
                )
            nc.sync.dma_start(bo_sb[:], bo_d[:])

            psq = [
                psqp.tile([128, 512], f32, tag=f"psq{i}", name=f"psq{i}")
                for i in range(8)
            ]
            mm_k0 = {}
            for k in range(16):
                wq_t = wq_g0 if k < 4 else wq_hi[k // 4 - 1]
                for m in range(16):
                    mm = nc.tensor.matmul(
                        psq[m // 2][:, (m % 2) * 256 : (m % 2) * 256 + 256],
                        lhsT=wq_t[:, k % 4, m * 128 : (m + 1) * 128],
                        rhs=hso[:, k, :],
                        start=(k == 0 and m % 2 == 0),
                        stop=(k == 15),
                        skip_group_check=True,
                    )
                    if k == 0:
                        mm_k0[m] = mm
                        if m % 2 == 1:
                            # the even-m k==0 matmul's start=True clears the
                            # whole bank's has_written bits; the odd-m k==0
                            # matmul must run after it (order-only dep)
                            tile.add_dep_helper(
                                mm.ins,
                                mm_k0[m - 1].ins,
                                sync=False,
                                reason="psq half-bank: odd k0 after even k0",
                            )
            for m in range(16):
                nc.vector.tensor_add(
                    qT[:, m, :],
                    psq[m // 2][:, (m % 2) * 256 : (m % 2) * 256 + 256],
                    bcf(bq_sb[:, m : m + 1], Q),
                )

            # ---- RoPE on q (rows 64:128), 4-head chunks; rotate-copies on
            # ACT so early heads unblock attention quickly ----
            for hb in range(4):
                hs_ = slice(hb * 4, (hb + 1) * 4)
                rq = rotq[:, (hb % 2) * 4 : (hb % 2) * 4 + 4, :]
                nc.scalar.copy(rq[96:128, :, :], qT[64:96, hs_, :])
                nc.scalar.copy(rq[64:96, :, :], qT[96:128, hs_, :])
                nc.vector.tensor_mul(
                    qT[64:128, hs_, :], qT[64:128, hs_, :], bc(tqc[64:128, :], 4)
                )
                nc.vector.tensor_mul(
                    rq[64:128, :, :], rq[64:128, :, :], bc(tqs[64:128, :], 4)
                )
                nc.vector.tensor_sub(
                    qT[64:96, hs_, :], qT[64:96, hs_, :], rq[64:96, :, :]
                )
                nc.vector.tensor_add(
                    qT[96:128, hs_, :], qT[96:128, hs_, :], rq[96:128, :, :]
                )

        # "ones" columns of v = key-validity mask, one 4D broadcast copy
        vones_view = v_sb[:].rearrange("p j (h d) -> p j h d", d=2 * V)[
            :, :, :, V : 2 * V
        ]
        kap = kmk[:]
        kmk_bcast = bass.AP(
            kap.tensor, kap.offset, [kap.ap[0], [1, NJB], [0, H], [0, V]]
        )
        nc.gpsimd.tensor_copy(vones_view, kmk_bcast)

        # ---- phase KN: k_nope into kf rows 0:64 (ACT), rope rows broadcast
        # into rows 64:128 (DVE; a K-split pair of row-group matmuls
        # accumulating into one PSUM region hard-crashes the device, so the
        # shared rope rows must be materialized per head) ----
        with tc.tile_pool(name="pskn", bufs=4, space="PSUM") as psknp:
            for m in range(8):
                ps = psknp.tile([128, KH], f32, tag="pskn")
                for k in range(4):
                    for n0, n1 in ((0, 512), (512, KH)):
                        nc.tensor.matmul(
                            ps[:, n0:n1],
                            lhsT=wkc[:, k, m * 128 : (m + 1) * 128],
                            rhs=latbf[:, k, n0:n1],
                            start=(k == 0),
                            stop=(k == 3),
                        )
                # PSUM->SBUF halves split across ACT and DVE; the shared rope
                # rows ride the (otherwise idle) DMA engines, SBUF->SBUF
                nc.scalar.copy(kf[0:64, 2 * m, :], ps[0:64, :])
                nc.vector.tensor_copy(kf[0:64, 2 * m + 1, :], ps[64:128, :])
                nc.sync.dma_start(kf[64:128, 2 * m, :], lat4b[64:128, :])
                nc.sync.dma_start(kf[64:128, 2 * m + 1, :], lat4b[64:128, :])

        # ---- phase V: V (key-major) ----
        with tc.tile_pool(name="psv", bufs=2, space="PSUM") as psvp:
            for jb in range(NJB):
                ps = psvp.tile([128, H * V], f32, tag="psv")
                for k in range(4):
                    for n0, n1 in ((0, 512), (512, 1024)):
                        nc.tensor.matmul(
                            ps[:, n0:n1],
                            lhsT=latbf[:, k, jb * 128 : (jb + 1) * 128],
                            rhs=wvc[:, k, n0:n1],
                            start=(k == 0),
                            stop=(k == 3),
                        )
                vview = v_sb[:, jb, :].rearrange("p (h d) -> p h d", d=2 * V)
                ps_view = ps[:].rearrange("p (h d) -> p h d", d=V)
                if jb % 2 == 0:
                    nc.scalar.copy(vview[:, :, 0:V], ps_view)
                else:
                    nc.vector.tensor_copy(vview[:, :, 0:V], ps_view)

        # ---- phase A: attention, packed 1536-col score layout ----
        # Score slots (cols): jb0 -> [0:128] (query tile 0 only; the it1 half
        # is uniformly outside the window), jb1..4 -> [jb*256 : jb*256+256]
        # (both query tiles), jb5 -> [1408:1536] (query tile 1 only).
        # Each block's score = two row-group-concurrent K=64 matmuls:
        # nope (kfn, rows 0:64) + shared rope (lat4b, rows 64:128).
        # Static masks: M2 (p>c, window edge) on cols {0,384}; M1 (p<=c,
        # causal edge) on cols {1024,1408} — identical for every core/head.
        probs_tiles = {}
        with tc.tile_pool(name="att_sbuf", bufs=2) as attp, tc.tile_pool(
            name="att_psum", bufs=2, space="PSUM"
        ) as attps, tc.tile_pool(name="stat", bufs=2) as statp:

            def sc_block(ps_s, h, jb, c0, q0, qn):
                return nc.tensor.matmul(
                    ps_s[:, c0 : c0 + qn],
                    lhsT=kf[:, h, jb * 128 : (jb + 1) * 128],
                    rhs=qT[:, h, q0 : q0 + qn],
                    start=True,
                    stop=True,
                )

            def emit_scores(h):
                ps_s = attps.tile([128, PACK], f32, tag="ps_s")
                sc_block(ps_s, h, 0, 0, 0, 128)
                for jb in range(1, 5):
                    sc_block(ps_s, h, jb, jb * 256, 0, 256)
                sc_block(ps_s, h, 5, 1408, 128, 128)
                pr = attp.tile([128, PACK], f16, tag="pr", bufs=3)
                # one exp per head: ACT runs back-to-back (throughput-bound)
                # in this window, so a split's earlier-start buys nothing and
                # its per-op overhead costs ~200ns/head (dead slots hold
                # stale PSUM garbage whose exp is never consumed)
                nc.scalar.activation(pr[:], ps_s[:], AF.Exp, bias=0.0, scale=SCALE)
                nc.vector.tensor_mul(
                    pair(pr, 0, 384, 128), pair(pr, 0, 384, 128), bc(m1m2[:, 1, :], 2)
                )
                nc.vector.tensor_mul(
                    pair(pr, 1024, 384, 128),
                    pair(pr, 1024, 384, 128),
                    bc(m1m2[:, 0, :], 2),
                )
                probs_tiles[h] = pr

            def emit_pv(h):
                pr = probs_tiles.pop(h)
                ps_o = attps.tile([128, Q], f32, tag="ps_o")
                nc.tensor.matmul(
                    ps_o[:, 0:128],
                    lhsT=v_sb[:, 0, h * 2 * V : (h + 1) * 2 * V],
                    rhs=pr[:, 0:128],
                    start=True,
                    stop=False,
                    skip_group_check=True,
                )
                for jb in range(1, 5):
                    nc.tensor.matmul(
                        ps_o[:],
                        lhsT=v_sb[:, jb, h * 2 * V : (h + 1) * 2 * V],
                        rhs=pr[:, jb * 256 : jb * 256 + 256],
                        start=False,
                        stop=False,
                        skip_group_check=True,
                    )
                nc.tensor.matmul(
                    ps_o[:, 128:256],
                    lhsT=v_sb[:, 5, h * 2 * V : (h + 1) * 2 * V],
                    rhs=pr[:, 1408:1536],
                    start=False,
                    stop=True,
                    skip_group_check=True,
                )
                # denominator for this head into the pair tile (even head in
                # partitions 0:64, odd in 64:128, mirroring oat's layout)
                base = (h % 2) * 64
                if h % 2 == 0:
                    pair_state["ds"] = statp.tile(
                        [128, Q], f32, tag="dsum", name=f"dsp{h}"
                    )
                dspair = pair_state["ds"]
                nc.vector.tensor_add(
                    dspair[base : base + 64, :],
                    ps_o[64:128, :],
                    bcf(esink_sb[base : base + 64, h : h + 1], Q),
                )
                ps_pair[h % 2] = ps_o
                if h % 2 == 1:
                    # pairwise normalize: rcp = exp(-ln(d)) on ACT (the DVE
                    # reciprocal costs ~6.4ns/element; the two table lookups
                    # are ~4x cheaper and ACT has the headroom here)
                    lnd = statp.tile([128, Q], f32, tag="lnd")
                    nc.scalar.activation(lnd[:], dspair[:], AF.Ln)
                    rcp = statp.tile([128, Q], f32, tag="rcp")
                    nc.scalar.activation(rcp[:], lnd[:], AF.Exp, scale=-1.0)
                    nc.vector.tensor_mul(
                        oat[0:64, (h - 1) // 2, :],
                        ps_pair[0][0:64, :],
                        rcp[0:64, :],
                    )
                    nc.vector.tensor_mul(
                        oat[64:128, h // 2, :], ps_pair[1][0:64, :], rcp[64:128, :]
                    )
                if DEBUG:
                    nc.sync.dma_start(dbg["pr"][:, h, :], pr[:])

            ps_pair = {}
            pair_state = {}
            emit_scores(0)
            emit_scores(1)
            for h in range(2, H):
                emit_scores(h)
                emit_pv(h - 2)
            emit_pv(H - 2)
            emit_pv(H - 1)

        # ---- phase O: output projection (i-major) + bias + store; query
        # tile 0 finishes (and its output DMAs start) while tile 1's matmuls
        # are still streaming ----
        with tc.tile_pool(name="psf", bufs=1, space="PSUM") as psfp:
            psf = [
                psfp.tile([128, 512], f32, tag=f"psf{i}", name=f"psf{i}")
                for i in range(8)
            ]
            for it in range(NIT):
                for k in range(8):
                    for n in range(4):
                        nc.tensor.matmul(
                            psf[it * 4 + n][:],
                            lhsT=oat[:, k, it * 128 : (it + 1) * 128],
                            rhs=wo_sb[:, k, n * 512 : (n + 1) * 512],
                            start=(k == 0),
                            stop=(k == 7),
                        )
                for n in range(4):
                    nc.vector.tensor_add(
                        out_sb[:, it, n * 512 : (n + 1) * 512],
                        psf[it * 4 + n][:],
                        bo_sb[:, n * 512 : (n + 1) * 512],
                    )
                    # SWDGE (casts f16 -> f32 inline): first (and only) DMA on
                    # each SW queue, so the ring entry carries one wait.
                    nc.gpsimd.dma_start(
                        out_d[it * 128 : (it + 1) * 128, n * 512 : (n + 1) * 512],
                        out_sb[:, it, n * 512 : (n + 1) * 512],
                    )

        if DEBUG:
            nc.sync.dma_start(dbg["lat"][:], latbf[:])
            nc.sync.dma_start(dbg["lat4"][:], lat4b[64:128, :])
            nc.sync.dma_start(dbg["q"][:], qT[:])
            nc.sync.dma_start(dbg["kf"][:], kf[:])
            nc.sync.dma_start(dbg["v"][:], v_sb[:])
            nc.sync.dma_start(dbg["oat"][:], oat[:])

    if not bool(int(os.environ.get("BASSNOSPLIT", "0"))):
        _split_multi_waits(nc, mybir)
    nc.finalize()
    return nc


def _split_multi_waits(nc, mybir):
    """The TPB ISA has a single embedded wait slot per instruction and this
    toolchain's walrus pass list has no wait-splitting pass ("Too many sync
    wait commands"). Hoist all-but-one wait of every multi-wait compute
    instruction into standalone same-engine EventSemaphore instructions
    placed immediately before it. HWDGE (SP/ACT-issued) DMAs are fair game
    too: their waits execute on the issuing sequencer before descriptor
    generation, so a preceding same-engine EventSemaphore is semantically
    identical. SWDGE (Pool) ring entries can't be split — assert those."""
    seq_ok = (mybir.InstEventSemaphore,)
    hwdge = (mybir.EngineType.SP, mybir.EngineType.Activation)
    n = 0
    for fn in nc.m.functions:
        for blk in fn.blocks:
            out = []
            for inst in blk.instructions:
                si = inst.sync_info
                if si is not None and len(si.on_wait) > 1 and not isinstance(inst, seq_ok):
                    if isinstance(inst, mybir.InstDMACopy) and inst.engine not in hwdge:
                        raise AssertionError(
                            f"DMA {inst.name} on {inst.engine} has "
                            f"{len(si.on_wait)} waits; SWDGE DMAs must carry "
                            "at most one"
                        )
                    for w in si.on_wait[:-1]:
                        n += 1
                        out.append(
                            mybir.InstEventSemaphore(
                                name=f"I-wsplit-{n}",
                                engine=inst.engine,
                                ins=[],
                                outs=[],
                                sync_info=mybir.SyncInfo(on_wait=[w], on_update=[]),
                            )
                        )
                    inst.sync_info = mybir.SyncInfo(
                        on_wait=[si.on_wait[-1]], on_update=si.on_update
                    )
                out.append(inst)
            blk.instructions = out
    return n


def _shuffle128(a):
    """[K*128, F] -> [128, K*F] with [p, k*F+f] = a[k*128+p, f]."""
    k = a.shape[0] // 128
    return np.ascontiguousarray(
        a.reshape(k, 128, a.shape[1]).transpose(1, 0, 2).reshape(128, -1)
    )


def prep_inputs(
    hidden_states, cos, sin, Wq, bq, Wo, bo, Wkva, bkva, w_kc, w_vc, sinks
):
    """Build the 8 per-core input dicts (numpy, fp16/fp32)."""
    f16 = np.float16
    hs = np.asarray(hidden_states, np.float32)[0]  # [S, HID]
    cos = np.asarray(cos, np.float32)[0]  # [S, ROPE]
    sin = np.asarray(sin, np.float32)[0]

    wqT = np.asarray(Wq, np.float32).T.astype(f16)
    wkvaT = np.asarray(Wkva, np.float32).T.astype(f16)
    wkc_p = np.asarray(w_kc, np.float32).transpose(2, 0, 1).reshape(R, H * NOPE).astype(f16)
    wvc_p = np.asarray(w_vc, np.float32).transpose(1, 0, 2).reshape(R, H * V).astype(f16)
    woT = np.asarray(Wo, np.float32).T.astype(f16)

    bq_t = np.ascontiguousarray(np.asarray(bq, np.float32).reshape(16, 128).T)
    bkva_pad = np.zeros(640, np.float32)
    bkva_pad[: R + ROPE] = np.asarray(bkva, np.float32)
    bkva_t = np.ascontiguousarray(bkva_pad.reshape(5, 128).T)
    bo_b = np.ascontiguousarray(
        np.broadcast_to(np.asarray(bo, np.float32), (128, HID))
    ).astype(f16)
    esink_b = np.ascontiguousarray(
        np.broadcast_to(np.exp(np.asarray(sinks, np.float32))[None, :], (128, H))
    )

    # static triangular edge masks: M1 = p<=c (causal), M2 = p>c (window)
    pp = np.arange(128)[:, None]
    cc = np.arange(128)[None, :]
    m1m2 = np.zeros((128, 2, 128), np.float32)
    m1m2[:, 0, :] = (pp <= cc).astype(np.float32)
    m1m2[:, 1, :] = (pp > cc).astype(np.float32)
    m1m2 = m1m2.astype(f16)

    hs_pad = np.zeros((SW + S, HID), np.float32)
    hs_pad[SW:] = hs

    shared = dict(
        wql=_shuffle128(wqT),
        wkval=_shuffle128(wkvaT),
        wkcl=_shuffle128(wkc_p),
        wvcl=_shuffle128(wvc_p),
        wol=_shuffle128(woT),
        bq=bq_t, bkva=bkva_t, bo=bo_b, esink=esink_b, m1m2=m1m2,
    )

    in_maps = []
    for c in range(NCORES):
        g0 = c * Q
        hsTh_c = np.ascontiguousarray(hs_pad[g0 : g0 + SW].T).astype(f16)
        hsTo_c = np.ascontiguousarray(hs_pad[g0 + SW : g0 + KH].T).astype(f16)

        cq = cos[g0 : g0 + Q]  # [Q, 64]
        sq = sin[g0 : g0 + Q]
        tqc = np.zeros((128, Q), np.float32)
        tqs = np.zeros((128, Q), np.float32)
        tqc[64:96] = cq[:, 0:32].T
        tqc[96:128] = cq[:, 32:64].T
        tqs[64:96] = sq[:, 0:32].T
        tqs[96:128] = sq[:, 32:64].T

        kpos = np.clip(np.arange(g0 - SW, g0 + Q), 0, None)
        ck = cos[kpos]  # [KH, 64]
        sk = sin[kpos]
        tkk = np.zeros((128, 2, KH), np.float32)
        tkk[64:96, 0] = ck[:, 0:32].T
        tkk[96:128, 0] = ck[:, 32:64].T
        tkk[64:96, 1] = sk[:, 0:32].T
        tkk[96:128, 1] = sk[:, 32:64].T

        # key validity (sequence start padding)
        jg = (g0 - SW) + np.arange(KH)
        kmr_c = np.broadcast_to((jg[0:SW] >= 0).astype(np.float32), (128, SW))
        kmk_c = np.zeros((128, NJB), np.float32)
        for jb in range(NJB):
            kmk_c[:, jb] = (jg[jb * 128 : (jb + 1) * 128] >= 0).astype(np.float32)

        in_maps.append(
            dict(
                shared,
                hshl=_shuffle128(hsTh_c),
                hsol=_shuffle128(hsTo_c),
                trigq_cos=tqc.astype(f16),
                trigq_sin=tqs.astype(f16),
                trigk=tkk.astype(f16),
                kmr=np.ascontiguousarray(kmr_c).astype(f16),
                kmk=kmk_c.astype(f16),
            )
        )
    return in_maps


def get_program():
    if "nc" not in _CACHE:
        _CACHE["nc"] = _build_program()
    return _CACHE["nc"]


def run(in_maps, **kw):
    from concourse.bass_utils import run_bass_kernel_spmd

    nc = get_program()
    return run_bass_kernel_spmd(nc, in_maps, list(range(NCORES)), **kw)


def kernel(**inputs):
    in_maps = prep_inputs(**inputs)
    res = run(in_maps)
    out = np.concatenate([res.results[c]["out"] for c in range(NCORES)], axis=0)
    return out.reshape(B, S, HID).astype(np.float32)


# revision 68
# speedup vs baseline: 1.1527x; 1.0676x over previous
"""Trainium2 Bass kernel for CareGptOssAttentionHF (MLA-style sliding-window
attention with sinks).

Sharding: sequence-parallel across 8 NeuronCores. Core c owns query rows
[c*256, (c+1)*256) and redundantly computes latent/K/V for its 768-row key
halo [c*256-512, c*256+256) — no collectives needed (window = 512).

v3 structure (v1 262us, v2 221us):
  * Every large input is host-pre-shuffled to a [128, N] layout whose
    per-partition bytes are contiguous in DRAM (8-16KB chunks), so DMAs
    run at line rate.
  * Phase order: latent -> k_nope -> V -> q-proj -> attention -> o-proj.
    kf/v live in fresh SBUF (pool opened first), so K/V assembly overlaps
    the Wq-gated q projection instead of serializing after it.
  * Packed 1536-col score layout per head; the two uniformly-dead
    (query-tile, key-block) pairs are never computed; causal/window edges
    are two static 128x128 triangles applied as 2 paired DVE multiplies.
  * Sequence-start (pad) keys: latent columns zeroed, and the PV "ones"
    column is the key-validity mask, so pad keys contribute exactly zero
    to numerator and denominator.
  * Softmax denominator reciprocal batched 8 heads per DVE op (the native
    reciprocal has ~1.4us fixed cost); numerators stashed in oat and
    normalized in place.
"""

import os
import sys

import numpy as np

if "/opt/trn_rl_repo" not in sys.path:
    sys.path.insert(0, "/opt/trn_rl_repo")

B, S, HID, H = 1, 2048, 2048, 16
NOPE = ROPE = 64
D = NOPE + ROPE  # 128
V = 64
R = 512
SW = 512
NCORES = 8
Q = S // NCORES  # 256 query rows per core
KH = Q + SW  # 768 halo key rows per core
NJB = KH // 128  # 6 key blocks
NIT = Q // 128  # 2 query tiles
SCALE = float(D) ** -0.5
PACK = 1536  # packed score cols: 6 blocks x 256-col slots
DEBUG = bool(int(os.environ.get("BASSDBG", "0")))

_CACHE = {}


def _build_program():
    import concourse.bass as bass
    import concourse.mybir as mybir
    from concourse import tile
    from contextlib import ExitStack

    f32 = mybir.dt.float32
    f16 = mybir.dt.float16
    AF = mybir.ActivationFunctionType

    nc = bass.Bass()

    # [128, N] host-pre-shuffled layouts: element [p, k*F + f] = orig[k*128+p, f]
    hsh_d = nc.dram_tensor("hshl", [128, 16 * SW], f16, kind="ExternalInput")
    hso_d = nc.dram_tensor("hsol", [128, 16 * Q], f16, kind="ExternalInput")
    wq_d = nc.dram_tensor("wql", [128, 16 * H * D], f16, kind="ExternalInput")
    wkva_d = nc.dram_tensor("wkval", [128, 16 * (R + ROPE)], f16, kind="ExternalInput")
    wkc_d = nc.dram_tensor("wkcl", [128, 4 * H * NOPE], f16, kind="ExternalInput")
    wvc_d = nc.dram_tensor("wvcl", [128, 4 * H * V], f16, kind="ExternalInput")
    wo_d = nc.dram_tensor("wol", [128, 8 * HID], f16, kind="ExternalInput")
    bq_d = nc.dram_tensor("bq", [128, 16], f32, kind="ExternalInput")
    bkva_d = nc.dram_tensor("bkva", [128, 5], f32, kind="ExternalInput")
    bo_d = nc.dram_tensor("bo", [128, HID], f16, kind="ExternalInput")
    tqc_d = nc.dram_tensor("trigq_cos", [128, Q], f16, kind="ExternalInput")
    tqs_d = nc.dram_tensor("trigq_sin", [128, Q], f16, kind="ExternalInput")
    tk_d = nc.dram_tensor("trigk", [128, 2, KH], f16, kind="ExternalInput")
    m1m2_d = nc.dram_tensor("m1m2", [128, 2, 128], f16, kind="ExternalInput")
    kmr_d = nc.dram_tensor("kmr", [128, SW], f16, kind="ExternalInput")
    kmk_d = nc.dram_tensor("kmk", [128, NJB], f16, kind="ExternalInput")
    esink_d = nc.dram_tensor("esink", [128, H], f32, kind="ExternalInput")
    out_d = nc.dram_tensor("out", [Q, HID], f32, kind="ExternalOutput")

    dbg = {}
    if DEBUG:
        dbg["lat"] = nc.dram_tensor("dbg_lat", [128, 4, KH], f16, kind="ExternalOutput")
        dbg["lat4"] = nc.dram_tensor("dbg_lat4", [64, KH], f16, kind="ExternalOutput")
        dbg["q"] = nc.dram_tensor("dbg_q", [128, H, Q], f16, kind="ExternalOutput")
        dbg["kf"] = nc.dram_tensor("dbg_kf", [128, H, KH], f16, kind="ExternalOutput")
        dbg["v"] = nc.dram_tensor("dbg_v", [128, NJB, H * 2 * V], f16, kind="ExternalOutput")
        dbg["pr"] = nc.dram_tensor("dbg_pr", [128, H, PACK], f16, kind="ExternalOutput")
        dbg["oat"] = nc.dram_tensor("dbg_oat", [128, 8, Q], f16, kind="ExternalOutput")

    with tile.TileContext(nc) as tc, ExitStack() as ctx:
        const = ctx.enter_context(tc.tile_pool(name="const", bufs=1))

        # ---- resident const tiles ----
        hso = const.tile([128, 16, Q], f16)
        wkc = const.tile([128, 4, H * NOPE], f16)
        wvc = const.tile([128, 4, H * V], f16)
        wo_sb = const.tile([128, 8, HID], f16)  # DMA deferred until after Wq
        bq_sb = const.tile([128, 16], f32)
        bkva_sb = const.tile([128, 5], f32)
        m1m2 = const.tile([128, 2, 128], f16)
        kmk = const.tile([128, NJB], f16)
        esink_sb = const.tile([128, H], f32)

        qT = const.tile([128, H, Q], f16)
        lat4b = const.tile([128, KH], f16)  # rows 64:128 = k_rope (RoPE'd)
        oat = const.tile([128, 8, Q], f16)
        # const (never-released) region: the output DMAs must carry only
        # their DVE-producer wait, so out_sb can't sit on reused space
        out_sb = const.tile([128, NIT, HID], f16)

        # kf/v/bo in fresh space, pool opened before the (short-lived) latent
        # pools so K/V assembly never waits on a region release
        kvp = ctx.enter_context(tc.tile_pool(name="kv", bufs=1))
        kf = kvp.tile([128, H, KH], f16)
        v_sb = kvp.tile([128, NJB, H * 2 * V], f16)
        bo_sb = kvp.tile([128, HID], f16)
        midp = ctx.enter_context(tc.tile_pool(name="mid", bufs=1))
        latbf = midp.tile([128, 4, KH], f16)
        # first half of Wq (k=0..7) in fresh space so its DMA starts
        # immediately; the rest reuses the latent-weights region
        wqp1 = ctx.enter_context(tc.tile_pool(name="wqp1", bufs=1))
        wq_g0 = wqp1.tile([128, 4, H * D], f16)
        wq_g1 = wqp1.tile([128, 4, H * D], f16)

        def bc(ap, n):
            # broadcast a [P, F] AP to [P, n, F] via a step-0 middle dim
            return bass.AP(ap.tensor, ap.offset, [ap.ap[0], [0, n], ap.ap[1]])

        def bcf(col, n):
            # broadcast a [P, 1] column AP to [P, n] via a step-0 free dim
            return bass.AP(col.tensor, col.offset, [col.ap[0], [0, n]])

        def pair(t2d, col0, stride, width):
            # [P, 2, width] view of a [P, F] tile: cols {col0, col0+stride}
            s = t2d[:, col0 : col0 + width]
            return bass.AP(s.tensor, s.offset, [s.ap[0], [stride, 2], s.ap[1]])

        # ---- phase L: latent projection ----
        with tc.tile_pool(name="wkvap", bufs=1) as wkvap:
            wkva = wkvap.tile([128, 16, R + ROPE], f16)
            hsh = wkvap.tile([128, 16, SW], f16)
            tk = wkvap.tile([128, 2, KH], f16)  # rows 64:128 = key cos/sin
            kmr = wkvap.tile([128, SW], f16)
            rotk = wkvap.tile([128, KH], f16)  # rows 64:128 scratch
            for g in range(2):
                nc.sync.dma_start(
                    wkva[:, g * 8 : (g + 1) * 8, :],
                    wkva_d[:, g * 8 * 576 : (g + 1) * 8 * 576].rearrange(
                        "p (k f) -> p k f", f=576
                    ),
                )
                nc.sync.dma_start(
                    hsh[:, g * 8 : (g + 1) * 8, :],
                    hsh_d[:, g * 8 * SW : (g + 1) * 8 * SW].rearrange(
                        "p (k f) -> p k f", f=SW
                    ),
                )
            nc.sync.dma_start(hso[:], hso_d[:].rearrange("p (k f) -> p k f", f=Q))
            nc.sync.dma_start(bkva_sb[:], bkva_d[:])
            nc.sync.dma_start(tk[:], tk_d[:])
            nc.sync.dma_start(kmr[:], kmr_d[:])
            nc.sync.dma_start(kmk[:], kmk_d[:])
            nc.sync.dma_start(bq_sb[:], bq_d[:])
            nc.sync.dma_start(m1m2[:], m1m2_d[:])
            nc.sync.dma_start(esink_sb[:], esink_d[:])
            # Wq first half streams in right behind the latent inputs
            nc.sync.dma_start(wq_g0[:], wq_d[:, 0 : 4 * H * D])
            nc.sync.dma_start(wq_g1[:], wq_d[:, 4 * H * D : 8 * H * D])
            nc.sync.dma_start(wkc[:], wkc_d[:].rearrange("p (k f) -> p k f", f=H * NOPE))
            nc.sync.dma_start(wvc[:], wvc_d[:].rearrange("p (k f) -> p k f", f=H * V))

            with tc.tile_pool(name="pslat", bufs=1, space="PSUM") as pslatp:
                pslat = [
                    pslatp.tile([128, KH], f32, tag=f"pslat{m}", name=f"pslat{m}")
                    for m in range(4)
                ]
                for k in range(16):
                    for m in range(4):
                        nc.tensor.matmul(
                            pslat[m][:, 0:SW],
                            lhsT=wkva[:, k, m * 128 : (m + 1) * 128],
                            rhs=hsh[:, k, :],
                            start=(k == 0),
                            stop=(k == 15),
                        )
                        nc.tensor.matmul(
                            pslat[m][:, SW:KH],
                            lhsT=wkva[:, k, m * 128 : (m + 1) * 128],
                            rhs=hso[:, k, :],
                            start=(k == 0),
                            stop=(k == 15),
                        )
                ps4 = pslatp.tile([64, KH], f32, tag="pslat0")
                for k in range(16):
                    nc.tensor.matmul(
                        ps4[:, 0:SW],
                        lhsT=wkva[:, k, 512:576],
                        rhs=hsh[:, k, :],
                        start=(k == 0),
                        stop=(k == 15),
                    )
                    nc.tensor.matmul(
                        ps4[:, SW:KH],
                        lhsT=wkva[:, k, 512:576],
                        rhs=hso[:, k, :],
                        start=(k == 0),
                        stop=(k == 15),
                    )
                for m in range(4):
                    nc.vector.tensor_add(
                        latbf[:, m, :], pslat[m][:], bcf(bkva_sb[:, m : m + 1], KH)
                    )
                    # zero pad-key columns (sequence start): kills k_nope & v
                    nc.vector.tensor_mul(
                        latbf[:, m, 0:SW], latbf[:, m, 0:SW], kmr[:]
                    )
                # k_rope into partitions 64:128 (partition-shifted DVE add)
                # rope features 512:576 sit in rows 0:64 of bkva column 4
                nc.vector.tensor_add(
                    lat4b[64:128, :], ps4[:], bcf(bkva_sb[0:64, 4:5], KH)
                )
                nc.vector.tensor_mul(
                    lat4b[64:128, 0:SW], lat4b[64:128, 0:SW], kmr[64:128, :]
                )

            # ---- RoPE on k_rope (rows 64:128 of lat4b; tk rows 64:128) ----
            nc.vector.tensor_copy(rotk[64:96, :], lat4b[96:128, :])
            nc.vector.tensor_copy(rotk[96:128, :], lat4b[64:96, :])
            nc.vector.tensor_mul(lat4b[64:128, :], lat4b[64:128, :], tk[64:128, 0, :])
            nc.vector.tensor_mul(rotk[64:96, :], rotk[64:96, :], tk[64:96, 1, :])
            nc.vector.tensor_sub(lat4b[64:96, :], lat4b[64:96, :], rotk[64:96, :])
            nc.vector.tensor_mul(rotk[96:128, :], rotk[96:128, :], tk[96:128, 1, :])
            nc.vector.tensor_add(lat4b[96:128, :], lat4b[96:128, :], rotk[96:128, :])

        # ---- phase Q: q projection (before K/V assembly, so the rope-q and
        # kf/v copies on ACT/DVE hide under the KN/V matmul stream) ----
        # All 16 feature-major [128, 256] accumulators live as 8 [128, 512]
        # bank tiles (two halves each). has_written is cleared once per bank
        # (k==0, even m); the odd-m k==0 matmul then overwrites its untouched
        # half and every k>0 matmul accumulates.
        with tc.tile_pool(name="wqp2", bufs=1) as wqp2, tc.tile_pool(
            name="psq", bufs=1, space="PSUM"
        ) as psqp:
            wq_hi = [
                wqp2.tile([128, 4, H * D], f16, name=f"wq{2 + g}") for g in range(2)
            ]
            rotq = wqp2.tile([128, 8, Q], f16)
            tqc = wqp2.tile([128, Q], f16)
            tqs = wqp2.tile([128, Q], f16)
            nc.sync.dma_start(tqc[:], tqc_d[:])
            nc.sync.dma_start(tqs[:], tqs_d[:])
            for g in range(2):
                nc.sync.dma_start(
                    wq_hi[g][:],
                    wq_d[:, (2 + g) * 4 * H * D : (3 + g) * 4 * H * D],
                )
            # Wo + bo: queued behind Wq on the sync DGE
            for g in range(2):
                nc.sync.dma_start(
                    wo_sb[:, g * 4 : (g + 1) * 4, :],
                    wo_d[:, g * 4 * HID : (g + 1) * 4 * HID],
                )
            nc.sync.dma_start(bo_sb[:], bo_d[:])

            psq = [
                psqp.tile([128, 512], f32, tag=f"psq{i}", name=f"psq{i}")
                for i in range(8)
            ]
            mm_k0 = {}
            for k in range(16):
                if k < 4:
                    wq_t = wq_g0
                elif k < 8:
                    wq_t = wq_g1
                else:
                    wq_t = wq_hi[k // 4 - 2]
                for m in range(16):
                    mm = nc.tensor.matmul(
                        psq[m // 2][:, (m % 2) * 256 : (m % 2) * 256 + 256],
                        lhsT=wq_t[:, k % 4, m * 128 : (m + 1) * 128],
                        rhs=hso[:, k, :],
                        start=(k == 0 and m % 2 == 0),
                        stop=(k == 15),
                        skip_group_check=True,
                    )
                    if k == 0:
                        mm_k0[m] = mm
                        if m % 2 == 1:
                            # the even-m k==0 matmul's start=True clears the
                            # whole bank's has_written bits; the odd-m k==0
                            # matmul must run after it (order-only dep)
                            tile.add_dep_helper(
                                mm.ins,
                                mm_k0[m - 1].ins,
                                sync=False,
                                reason="psq half-bank: odd k0 after even k0",
                            )
            for m in range(16):
                nc.vector.tensor_add(
                    qT[:, m, :],
                    psq[m // 2][:, (m % 2) * 256 : (m % 2) * 256 + 256],
                    bcf(bq_sb[:, m : m + 1], Q),
                )

            # ---- RoPE on q (rows 64:128), 4-head chunks; rotate-copies on
            # ACT so early heads unblock attention quickly ----
            for hb in range(4):
                hs_ = slice(hb * 4, (hb + 1) * 4)
                rq = rotq[:, (hb % 2) * 4 : (hb % 2) * 4 + 4, :]
                nc.scalar.copy(rq[96:128, :, :], qT[64:96, hs_, :])
                nc.scalar.copy(rq[64:96, :, :], qT[96:128, hs_, :])
                nc.vector.tensor_mul(
                    qT[64:128, hs_, :], qT[64:128, hs_, :], bc(tqc[64:128, :], 4)
                )
                nc.vector.tensor_mul(
                    rq[64:128, :, :], rq[64:128, :, :], bc(tqs[64:128, :], 4)
                )
                nc.vector.tensor_sub(
                    qT[64:96, hs_, :], qT[64:96, hs_, :], rq[64:96, :, :]
                )
                nc.vector.tensor_add(
                    qT[96:128, hs_, :], qT[96:128, hs_, :], rq[96:128, :, :]
                )

        # "ones" columns of v = key-validity mask, one 4D broadcast copy
        vones_view = v_sb[:].rearrange("p j (h d) -> p j h d", d=2 * V)[
            :, :, :, V : 2 * V
        ]
        kap = kmk[:]
        kmk_bcast = bass.AP(
            kap.tensor, kap.offset, [kap.ap[0], [1, NJB], [0, H], [0, V]]
        )
        nc.gpsimd.tensor_copy(vones_view, kmk_bcast)

        # ---- phase KN: k_nope into kf rows 0:64 (ACT), rope rows broadcast
        # into rows 64:128 (DVE; a K-split pair of row-group matmuls
        # accumulating into one PSUM region hard-crashes the device, so the
        # shared rope rows must be materialized per head) ----
        with tc.tile_pool(name="pskn", bufs=4, space="PSUM") as psknp:
            for m in range(8):
                ps = psknp.tile([128, KH], f32, tag="pskn")
                for k in range(4):
                    for n0, n1 in ((0, 512), (512, KH)):
                        nc.tensor.matmul(
                            ps[:, n0:n1],
                            lhsT=wkc[:, k, m * 128 : (m + 1) * 128],
                            rhs=latbf[:, k, n0:n1],
                            start=(k == 0),
                            stop=(k == 3),
                        )
                # PSUM->SBUF halves split across ACT and DVE; the shared rope
                # rows ride the (otherwise idle) DMA engines, SBUF->SBUF
                nc.scalar.copy(kf[0:64, 2 * m, :], ps[0:64, :])
                nc.vector.tensor_copy(kf[0:64, 2 * m + 1, :], ps[64:128, :])
                nc.sync.dma_start(kf[64:128, 2 * m, :], lat4b[64:128, :])
                nc.sync.dma_start(kf[64:128, 2 * m + 1, :], lat4b[64:128, :])

        # ---- phase V: V (key-major) ----
        with tc.tile_pool(name="psv", bufs=2, space="PSUM") as psvp:
            for jb in range(NJB):
                ps = psvp.tile([128, H * V], f32, tag="psv")
                for k in range(4):
                    for n0, n1 in ((0, 512), (512, 1024)):
                        nc.tensor.matmul(
                            ps[:, n0:n1],
                            lhsT=latbf[:, k, jb * 128 : (jb + 1) * 128],
                            rhs=wvc[:, k, n0:n1],
                            start=(k == 0),
                            stop=(k == 3),
                        )
                vview = v_sb[:, jb, :].rearrange("p (h d) -> p h d", d=2 * V)
                ps_view = ps[:].rearrange("p (h d) -> p h d", d=V)
                if jb % 2 == 0:
                    nc.scalar.copy(vview[:, :, 0:V], ps_view)
                else:
                    nc.vector.tensor_copy(vview[:, :, 0:V], ps_view)

        # ---- phase A: attention, packed 1536-col score layout ----
        # Score slots (cols): jb0 -> [0:128] (query tile 0 only; the it1 half
        # is uniformly outside the window), jb1..4 -> [jb*256 : jb*256+256]
        # (both query tiles), jb5 -> [1408:1536] (query tile 1 only).
        # Each block's score = two row-group-concurrent K=64 matmuls:
        # nope (kfn, rows 0:64) + shared rope (lat4b, rows 64:128).
        # Static masks: M2 (p>c, window edge) on cols {0,384}; M1 (p<=c,
        # causal edge) on cols {1024,1408} — identical for every core/head.
        probs_tiles = {}
        with tc.tile_pool(name="att_sbuf", bufs=2) as attp, tc.tile_pool(
            name="att_psum", bufs=2, space="PSUM"
        ) as attps, tc.tile_pool(name="stat", bufs=2) as statp:

            def sc_block(ps_s, h, jb, c0, q0, qn):
                return nc.tensor.matmul(
                    ps_s[:, c0 : c0 + qn],
                    lhsT=kf[:, h, jb * 128 : (jb + 1) * 128],
                    rhs=qT[:, h, q0 : q0 + qn],
                    start=True,
                    stop=True,
                )

            def emit_scores(h):
                ps_s = attps.tile([128, PACK], f32, tag="ps_s")
                sc_block(ps_s, h, 0, 0, 0, 128)
                for jb in range(1, 5):
                    sc_block(ps_s, h, jb, jb * 256, 0, 256)
                sc_block(ps_s, h, 5, 1408, 128, 128)
                pr = attp.tile([128, PACK], f16, tag="pr", bufs=3)
                # exp in two bank-aligned halves: each half depends only on
                # its three score matmuls, so exp overlaps the score stream
                # and the ps_s banks free earlier (dead slots hold stale PSUM
                # garbage whose exp is never consumed)
                nc.scalar.activation(
                    pr[:, 0:768], ps_s[:, 0:768], AF.Exp, bias=0.0, scale=SCALE
                )
                nc.scalar.activation(
                    pr[:, 768:PACK], ps_s[:, 768:PACK], AF.Exp, bias=0.0, scale=SCALE
                )
                nc.vector.tensor_mul(
                    pair(pr, 0, 384, 128), pair(pr, 0, 384, 128), bc(m1m2[:, 1, :], 2)
                )
                nc.vector.tensor_mul(
                    pair(pr, 1024, 384, 128),
                    pair(pr, 1024, 384, 128),
                    bc(m1m2[:, 0, :], 2),
                )
                probs_tiles[h] = pr

            def emit_pv(h):
                pr = probs_tiles.pop(h)
                ps_o = attps.tile([128, Q], f32, tag="ps_o")
                nc.tensor.matmul(
                    ps_o[:, 0:128],
                    lhsT=v_sb[:, 0, h * 2 * V : (h + 1) * 2 * V],
                    rhs=pr[:, 0:128],
                    start=True,
                    stop=False,
                    skip_group_check=True,
                )
                for jb in range(1, 5):
                    nc.tensor.matmul(
                        ps_o[:],
                        lhsT=v_sb[:, jb, h * 2 * V : (h + 1) * 2 * V],
                        rhs=pr[:, jb * 256 : jb * 256 + 256],
                        start=False,
                        stop=False,
                        skip_group_check=True,
                    )
                nc.tensor.matmul(
                    ps_o[:, 128:256],
                    lhsT=v_sb[:, 5, h * 2 * V : (h + 1) * 2 * V],
                    rhs=pr[:, 1408:1536],
                    start=False,
                    stop=True,
                    skip_group_check=True,
                )
                # denominator for this head into the pair tile (even head in
                # partitions 0:64, odd in 64:128, mirroring oat's layout)
                base = (h % 2) * 64
                if h % 2 == 0:
                    pair_state["ds"] = statp.tile(
                        [128, Q], f32, tag="dsum", name=f"dsp{h}"
                    )
                dspair = pair_state["ds"]
                nc.vector.tensor_add(
                    dspair[base : base + 64, :],
                    ps_o[64:128, :],
                    bcf(esink_sb[base : base + 64, h : h + 1], Q),
                )
                ps_pair[h % 2] = ps_o
                if h % 2 == 1:
                    # pairwise normalize: rcp = exp(-ln(d)) on ACT (the DVE
                    # reciprocal costs ~6.4ns/element; the two table lookups
                    # are ~4x cheaper and ACT has the headroom here)
                    lnd = statp.tile([128, Q], f32, tag="lnd")
                    nc.scalar.activation(lnd[:], dspair[:], AF.Ln)
                    rcp = statp.tile([128, Q], f32, tag="rcp")
                    nc.scalar.activation(rcp[:], lnd[:], AF.Exp, scale=-1.0)
                    nc.vector.tensor_mul(
                        oat[0:64, (h - 1) // 2, :],
                        ps_pair[0][0:64, :],
                        rcp[0:64, :],
                    )
                    nc.vector.tensor_mul(
                        oat[64:128, h // 2, :], ps_pair[1][0:64, :], rcp[64:128, :]
                    )
                if DEBUG:
                    nc.sync.dma_start(dbg["pr"][:, h, :], pr[:])

            ps_pair = {}
            pair_state = {}
            emit_scores(0)
            emit_scores(1)
            for h in range(2, H):
                emit_scores(h)
                emit_pv(h - 2)
            emit_pv(H - 2)
            emit_pv(H - 1)

        # ---- phase O: output projection (i-major) + bias + store; query
        # tile 0 finishes (and its output DMAs start) while tile 1's matmuls
        # are still streaming ----
        with tc.tile_pool(name="psf", bufs=1, space="PSUM") as psfp:
            psf = [
                psfp.tile([128, 512], f32, tag=f"psf{i}", name=f"psf{i}")
                for i in range(8)
            ]
            for it in range(NIT):
                for k in range(8):
                    for n in range(4):
                        nc.tensor.matmul(
                            psf[it * 4 + n][:],
                            lhsT=oat[:, k, it * 128 : (it + 1) * 128],
                            rhs=wo_sb[:, k, n * 512 : (n + 1) * 512],
                            start=(k == 0),
                            stop=(k == 7),
                        )
                for n in range(4):
                    nc.vector.tensor_add(
                        out_sb[:, it, n * 512 : (n + 1) * 512],
                        psf[it * 4 + n][:],
                        bo_sb[:, n * 512 : (n + 1) * 512],
                    )
                    # SWDGE (casts f16 -> f32 inline): first (and only) DMA on
                    # each SW queue, so the ring entry carries one wait.
                    nc.gpsimd.dma_start(
                        out_d[it * 128 : (it + 1) * 128, n * 512 : (n + 1) * 512],
                        out_sb[:, it, n * 512 : (n + 1) * 512],
                    )

        if DEBUG:
            nc.sync.dma_start(dbg["lat"][:], latbf[:])
            nc.sync.dma_start(dbg["lat4"][:], lat4b[64:128, :])
            nc.sync.dma_start(dbg["q"][:], qT[:])
            nc.sync.dma_start(dbg["kf"][:], kf[:])
            nc.sync.dma_start(dbg["v"][:], v_sb[:])
            nc.sync.dma_start(dbg["oat"][:], oat[:])

    if not bool(int(os.environ.get("BASSNOSPLIT", "0"))):
        _split_multi_waits(nc, mybir)
    nc.finalize()
    return nc


def _split_multi_waits(nc, mybir):
    """The TPB ISA has a single embedded wait slot per instruction and this
    toolchain's walrus pass list has no wait-splitting pass ("Too many sync
    wait commands"). Hoist all-but-one wait of every multi-wait compute
    instruction into standalone same-engine EventSemaphore instructions
    placed immediately before it. HWDGE (SP/ACT-issued) DMAs are fair game
    too: their waits execute on the issuing sequencer before descriptor
    generation, so a preceding same-engine EventSemaphore is semantically
    identical. SWDGE (Pool) ring entries can't be split — assert those."""
    seq_ok = (mybir.InstEventSemaphore,)
    hwdge = (mybir.EngineType.SP, mybir.EngineType.Activation)
    n = 0
    for fn in nc.m.functions:
        for blk in fn.blocks:
            out = []
            for inst in blk.instructions:
                si = inst.sync_info
                if si is not None and len(si.on_wait) > 1 and not isinstance(inst, seq_ok):
                    if isinstance(inst, mybir.InstDMACopy) and inst.engine not in hwdge:
                        raise AssertionError(
                            f"DMA {inst.name} on {inst.engine} has "
                            f"{len(si.on_wait)} waits; SWDGE DMAs must carry "
                            "at most one"
                        )
                    for w in si.on_wait[:-1]:
                        n += 1
                        out.append(
                            mybir.InstEventSemaphore(
                                name=f"I-wsplit-{n}",
                                engine=inst.engine,
                                ins=[],
                                outs=[],
                                sync_info=mybir.SyncInfo(on_wait=[w], on_update=[]),
                            )
                        )
                    inst.sync_info = mybir.SyncInfo(
                        on_wait=[si.on_wait[-1]], on_update=si.on_update
                    )
                out.append(inst)
            blk.instructions = out
    return n


def _shuffle128(a):
    """[K*128, F] -> [128, K*F] with [p, k*F+f] = a[k*128+p, f]."""
    k = a.shape[0] // 128
    return np.ascontiguousarray(
        a.reshape(k, 128, a.shape[1]).transpose(1, 0, 2).reshape(128, -1)
    )


def prep_inputs(
    hidden_states, cos, sin, Wq, bq, Wo, bo, Wkva, bkva, w_kc, w_vc, sinks
):
    """Build the 8 per-core input dicts (numpy, fp16/fp32)."""
    f16 = np.float16
    hs = np.asarray(hidden_states, np.float32)[0]  # [S, HID]
    cos = np.asarray(cos, np.float32)[0]  # [S, ROPE]
    sin = np.asarray(sin, np.float32)[0]

    wqT = np.asarray(Wq, np.float32).T.astype(f16)
    wkvaT = np.asarray(Wkva, np.float32).T.astype(f16)
    wkc_p = np.asarray(w_kc, np.float32).transpose(2, 0, 1).reshape(R, H * NOPE).astype(f16)
    wvc_p = np.asarray(w_vc, np.float32).transpose(1, 0, 2).reshape(R, H * V).astype(f16)
    woT = np.asarray(Wo, np.float32).T.astype(f16)

    bq_t = np.ascontiguousarray(np.asarray(bq, np.float32).reshape(16, 128).T)
    bkva_pad = np.zeros(640, np.float32)
    bkva_pad[: R + ROPE] = np.asarray(bkva, np.float32)
    bkva_t = np.ascontiguousarray(bkva_pad.reshape(5, 128).T)
    bo_b = np.ascontiguousarray(
        np.broadcast_to(np.asarray(bo, np.float32), (128, HID))
    ).astype(f16)
    esink_b = np.ascontiguousarray(
        np.broadcast_to(np.exp(np.asarray(sinks, np.float32))[None, :], (128, H))
    )

    # static triangular edge masks: M1 = p<=c (causal), M2 = p>c (window)
    pp = np.arange(128)[:, None]
    cc = np.arange(128)[None, :]
    m1m2 = np.zeros((128, 2, 128), np.float32)
    m1m2[:, 0, :] = (pp <= cc).astype(np.float32)
    m1m2[:, 1, :] = (pp > cc).astype(np.float32)
    m1m2 = m1m2.astype(f16)

    hs_pad = np.zeros((SW + S, HID), np.float32)
    hs_pad[SW:] = hs

    shared = dict(
        wql=_shuffle128(wqT),
        wkval=_shuffle128(wkvaT),
        wkcl=_shuffle128(wkc_p),
        wvcl=_shuffle128(wvc_p),
        wol=_shuffle128(woT),
        bq=bq_t, bkva=bkva_t, bo=bo_b, esink=esink_b, m1m2=m1m2,
    )

    in_maps = []
    for c in range(NCORES):
        g0 = c * Q
        hsTh_c = np.ascontiguousarray(hs_pad[g0 : g0 + SW].T).astype(f16)
        hsTo_c = np.ascontiguousarray(hs_pad[g0 + SW : g0 + KH].T).astype(f16)

        cq = cos[g0 : g0 + Q]  # [Q, 64]
        sq = sin[g0 : g0 + Q]
        tqc = np.zeros((128, Q), np.float32)
        tqs = np.zeros((128, Q), np.float32)
        tqc[64:96] = cq[:, 0:32].T
        tqc[96:128] = cq[:, 32:64].T
        tqs[64:96] = sq[:, 0:32].T
        tqs[96:128] = sq[:, 32:64].T

        kpos = np.clip(np.arange(g0 - SW, g0 + Q), 0, None)
        ck = cos[kpos]  # [KH, 64]
        sk = sin[kpos]
        tkk = np.zeros((128, 2, KH), np.float32)
        tkk[64:96, 0] = ck[:, 0:32].T
        tkk[96:128, 0] = ck[:, 32:64].T
        tkk[64:96, 1] = sk[:, 0:32].T
        tkk[96:128, 1] = sk[:, 32:64].T

        # key validity (sequence start padding)
        jg = (g0 - SW) + np.arange(KH)
        kmr_c = np.broadcast_to((jg[0:SW] >= 0).astype(np.float32), (128, SW))
        kmk_c = np.zeros((128, NJB), np.float32)
        for jb in range(NJB):
            kmk_c[:, jb] = (jg[jb * 128 : (jb + 1) * 128] >= 0).astype(np.float32)

        in_maps.append(
            dict(
                shared,
                hshl=_shuffle128(hsTh_c),
                hsol=_shuffle128(hsTo_c),
                trigq_cos=tqc.astype(f16),
                trigq_sin=tqs.astype(f16),
                trigk=tkk.astype(f16),
                kmr=np.ascontiguousarray(kmr_c).astype(f16),
                kmk=kmk_c.astype(f16),
            )
        )
    return in_maps


def get_program():
    if "nc" not in _CACHE:
        _CACHE["nc"] = _build_program()
    return _CACHE["nc"]


def run(in_maps, **kw):
    from concourse.bass_utils import run_bass_kernel_spmd

    nc = get_program()
    return run_bass_kernel_spmd(nc, in_maps, list(range(NCORES)), **kw)


def kernel(**inputs):
    in_maps = prep_inputs(**inputs)
    res = run(in_maps)
    out = np.concatenate([res.results[c]["out"] for c in range(NCORES)], axis=0)
    return out.reshape(B, S, HID).astype(np.float32)
